# revision 1
# baseline (speedup 1.0000x reference)
"""Trainium2 Bass kernel for nn_CompressiveEncoder (4-layer relative-position
transformer encoder, B=4 S=1024 D=1024 H=16 FF=4096).

Sharding: 8 cores = (batch b = c//2) x (query-half q0 = (c%2)*512).
Each layer starts with an 8-core AllGather of the bf16 transposed hidden
state; each core selects its batch pair from the gathered buffer via an
indirect DMA driven by host-supplied per-core indices, computes K/V for its
full batch, and attention + FFN for its own 512 query rows.  Matmuls run in
bf16 with fp32 PSUM accumulation; the residual stream stays fp32.  The
Music-Transformer shift() is a strided skewed DMA read from a DRAM scratch
buffer; the per-core query offset q0 is folded into a host-side roll of the
relative-position tables so the program is core-independent (SPMD).
"""

import sys

sys.path.insert(0, "/opt/trn_rl_repo")

import numpy as np
import ml_dtypes

import concourse.bass as bass
import concourse.mybir as mybir
import concourse.tile as tile
from concourse import bacc
from concourse.bass_utils import run_bass_kernel_spmd

BF16 = mybir.dt.bfloat16
F32 = mybir.dt.float32
AF = mybir.ActivationFunctionType
ALU = mybir.AluOpType

NL, D, H, DH, S, FF_DIM = 4, 1024, 16, 64, 1024, 4096
B = 4
NCORES = 8
NQ = 512           # query rows per core
P = 128
SCALE = float(H) ** -0.5   # reference scales by 1/sqrt(heads) = 0.25
EPS = 1e-5
NQC = NQ // P      # 4
NDC = D // P       # 8
NFC = FF_DIM // P  # 32
SKW = 2048         # skew buffer row width (elements)

_cache: dict = {}


def _ap(t, off, pattern):
    return bass.AP(tensor=t.tensor, offset=t.offset + off, ap=pattern)


def build():
    nc = bacc.Bacc("TRN2", target_bir_lowering=False, debug=False,
                   num_devices=NCORES)

    xrow0 = nc.dram_tensor("xrow0", [NQ, D], F32, kind="ExternalInput")
    xt0 = nc.dram_tensor("xt0", [D, NQ], BF16, kind="ExternalInput")
    wproj = nc.dram_tensor("wproj", [NL, 3, NDC, P, D], BF16,
                           kind="ExternalInput")
    wot = nc.dram_tensor("wot", [NL, D, D], BF16, kind="ExternalInput")
    w1r = nc.dram_tensor("w1r", [NL, NFC, P, D], BF16, kind="ExternalInput")
    w2t = nc.dram_tensor("w2t", [NL, FF_DIM, D], BF16, kind="ExternalInput")
    b1r = nc.dram_tensor("b1r", [NL, P, NFC], F32, kind="ExternalInput")
    b2r = nc.dram_tensor("b2r", [NL, D], F32, kind="ExternalInput")
    repd = nc.dram_tensor("repd", [NL, NDC, P, S], BF16,
                          kind="ExternalInput")
    rbd = nc.dram_tensor("rbd", [NL, H, S], BF16, kind="ExternalInput")
    rwbr = nc.dram_tensor("rwbr", [NL, P, NDC], F32, kind="ExternalInput")
    ident_d = nc.dram_tensor("ident", [P, P], BF16, kind="ExternalInput")
    agidx = nc.dram_tensor("agidx", [P, 2 * NDC], mybir.dt.int32,
                           kind="ExternalInput")
    yout = nc.dram_tensor("y", [NQ, D], F32, kind="ExternalOutput")

    # internal DRAM (raw tensors; indirect-DMA source needs offset 0)
    sk = nc.dram_tensor("sk_buf", [H * NQ * SKW], BF16)
    agin = nc.dram_tensor("agin_buf", [D * NQ], BF16)
    agout = nc.dram_tensor("agout_buf", [NCORES * D, NQ], BF16)

    with tile.TileContext(nc) as tc:
        with (
            tc.tile_pool(name="singles", bufs=1) as singles,
            tc.tile_pool(name="wT", bufs=2) as wTp,
            tc.tile_pool(name="wrhs", bufs=9) as wrhsp,
            tc.tile_pool(name="gt", bufs=17) as gtp,
            tc.tile_pool(name="attn", bufs=4) as attnp,
            tc.tile_pool(name="attnT", bufs=2) as attnTp,
            tc.tile_pool(name="brawsb", bufs=2) as brawp,
            tc.tile_pool(name="bd", bufs=2) as bdp,
            tc.tile_pool(name="rep", bufs=2) as repp,
            tc.tile_pool(name="small", bufs=16) as smallp,
            tc.tile_pool(name="vtt", bufs=2) as vttp,
            tc.tile_pool(name="rb", bufs=2) as rbp,
            tc.tile_pool(name="xw", bufs=2) as xwp,
            tc.tile_pool(name="psum", bufs=2, space="PSUM") as psp,
        ):
            # ------------- persistent SBUF state -------------
            x_row = [singles.tile([P, D], F32, tag=f"xrow{i}", name=f"xrow{i}")
                     for i in range(NQC)]
            xTown = [singles.tile([P, NQ], BF16, tag=f"xto{i}", name=f"xto{i}")
                     for i in range(NDC)]       # my own x^T (this layer's in)
            xT = [singles.tile([P, 2 * NQ], BF16, tag=f"xt{i}", name=f"xt{i}")
                  for i in range(NDC)]          # gathered x^T, my full batch
            xT1 = [singles.tile([P, NQ], BF16, tag=f"xt1_{i}", name=f"xt1_{i}")
                   for i in range(NDC)]         # post-LN1 x^T, my rows
            kT = [singles.tile([P, S], BF16, tag=f"kt{i}", name=f"kt{i}")
                  for i in range(NDC)]
            vrow = [singles.tile([P, D], BF16, tag=f"vr{i}", name=f"vr{i}")
                    for i in range(NDC)]
            rq = [singles.tile([P, NQ], BF16, tag=f"rq{i}", name=f"rq{i}")
                  for i in range(NDC)]
            aoT = [singles.tile([P, NQ], BF16, tag=f"aoT{i}", name=f"aoT{i}")
                   for i in range(NDC)]
            ff2acc = [singles.tile([P, D], F32, tag=f"ff2{i}", name=f"ff2{i}")
                      for i in range(NQC)]
            ident = singles.tile([P, P], BF16, tag="ident", name="ident")
            eps_t = singles.tile([P, 1], F32, tag="eps", name="eps")
            zb_t = singles.tile([P, 1], F32, tag="zbias", name="zbias")
            b2_t = singles.tile([P, D], F32, tag="b2rep", name="b2rep")
            b1_t = singles.tile([P, NFC], F32, tag="b1", name="b1")
            rwb_t = singles.tile([P, NDC], F32, tag="rwb", name="rwb")
            agix_t = singles.tile([P, 2 * NDC], mybir.dt.int32, tag="agix", name="agix")
            zeros_t = singles.tile([P, 1024], BF16, tag="zeros", name="zeros")

            nc.sync.dma_start(ident[:], ident_d.ap())
            nc.sync.dma_start(agix_t[:], agidx.ap())
            nc.vector.memset(eps_t[:], EPS)
            nc.vector.memset(zb_t[:], 0.0)
            nc.vector.memset(zeros_t[:], 0.0)
            skap = sk.ap()
            for blk in range(H * NQ // P):   # zero skew pad halves (once)
                dst = _ap(skap, blk * P * SKW + 1024, [[SKW, P], [1, 1024]])
                nc.sync.dma_start(dst, zeros_t[:, :])
            for qc in range(NQC):
                nc.sync.dma_start(x_row[qc][:],
                                  xrow0.ap()[qc * P:(qc + 1) * P, :])
            for dc in range(NDC):
                nc.sync.dma_start(xTown[dc][:],
                                  xt0.ap()[dc * P:(dc + 1) * P, :])
            nc.sync.dma_start(agin.ap(), xt0.ap())

            def mm(out, lhsT, rhs, first=True, last=True):
                nc.tensor.matmul(out, lhsT, rhs, start=first, stop=last)

            def layernorm(xr):
                st = smallp.tile([P, 2, 6], F32, tag="bnst", name="bnst")
                nc.vector.bn_stats(st[:, 0, :], xr[:, 0:512])
                nc.vector.bn_stats(st[:, 1, :], xr[:, 512:1024])
                mv = smallp.tile([P, 2], F32, tag="bnmv", name="bnmv")
                nc.vector.bn_aggr(mv[:], st[:])
                sd = smallp.tile([P, 1], F32, tag="sd", name="sd")
                nc.scalar.activation(sd[:], mv[:, 1:2], AF.Sqrt,
                                     bias=eps_t[:], scale=1.0)
                rs = smallp.tile([P, 1], F32, tag="rs", name="rs")
                nc.vector.reciprocal(rs[:], sd[:])
                nc.vector.tensor_scalar(
                    out=xr[:], in0=xr[:], scalar1=mv[:, 0:1],
                    scalar2=rs[:], op0=ALU.subtract, op1=ALU.mult)

            def transpose_to(dsts, src_bf16, qc):
                """src [128(q), 1024(d)] -> dsts[dc][:, qc*128:+128]."""
                for g in range(2):
                    pt = psp.tile([P, 512], BF16, tag="mm", name="mm")
                    for k4 in range(4):
                        dc = g * 4 + k4
                        nc.tensor.transpose(
                            pt[:, k4 * P:(k4 + 1) * P],
                            src_bf16[:, dc * P:(dc + 1) * P], ident[:])
                    for k4 in range(4):
                        dc = g * 4 + k4
                        nc.vector.tensor_copy(
                            dsts[dc][:, qc * P:(qc + 1) * P],
                            pt[:, k4 * P:(k4 + 1) * P])

            for li in range(NL):
                # ===== allgather x^T; pick my batch pair =====
                nc.gpsimd.collective_compute(
                    "AllGather", ALU.bypass,
                    replica_groups=[list(range(NCORES))],
                    ins=[agin.ap()], outs=[agout.ap()],
                )
                for dc in range(NDC):
                    for half in range(2):
                        nc.gpsimd.indirect_dma_start(
                            out=xT[dc][:, half * NQ:(half + 1) * NQ],
                            out_offset=None,
                            in_=agout.ap(),
                            in_offset=bass.IndirectOffsetOnAxis(
                                ap=agix_t[:, dc * 2 + half:
                                          dc * 2 + half + 1],
                                axis=0,
                            ),
                        )

                nc.sync.dma_start(rwb_t[:], rwbr.ap()[li])
                nc.sync.dma_start(b1_t[:], b1r.ap()[li])
                nc.sync.dma_start(
                    b2_t[:], _ap(b2r.ap(), li * D, [[0, P], [1, D]]))

                # ===== q/k/v projections =====
                for oc in range(NDC):
                    wq = wTp.tile([P, D], BF16, tag="wq", name="wq")
                    nc.sync.dma_start(wq[:], wproj.ap()[li, 0, oc])
                    ps = psp.tile([P, NQ], F32, tag="mm", name="mm")
                    for dc in range(NDC):
                        mm(ps[:], wq[:, dc * P:(dc + 1) * P], xTown[dc][:],
                           first=(dc == 0), last=(dc == NDC - 1))
                    nc.vector.tensor_scalar(
                        out=rq[oc][:], in0=ps[:],
                        scalar1=rwb_t[:, oc:oc + 1],
                        scalar2=None, op0=ALU.add)

                for oc in range(NDC):
                    wk = wTp.tile([P, D], BF16, tag="wq", name="wq")
                    nc.sync.dma_start(wk[:], wproj.ap()[li, 1, oc])
                    psk = psp.tile([P, S], F32, tag="score", name="score")
                    for jh in range(2):
                        for dc in range(NDC):
                            mm(psk[:, jh * 512:(jh + 1) * 512],
                               wk[:, dc * P:(dc + 1) * P],
                               xT[dc][:, jh * 512:(jh + 1) * 512],
                               first=(dc == 0), last=(dc == NDC - 1))
                    nc.vector.tensor_copy(kT[oc][:], psk[:])

                for oc in range(NDC):
                    wv = wTp.tile([P, D], BF16, tag="wq", name="wq")
                    nc.sync.dma_start(wv[:], wproj.ap()[li, 2, oc])
                    psv = psp.tile([P, S], F32, tag="score", name="score")
                    for jh in range(2):
                        for dc in range(NDC):
                            mm(psv[:, jh * 512:(jh + 1) * 512],
                               wv[:, dc * P:(dc + 1) * P],
                               xT[dc][:, jh * 512:(jh + 1) * 512],
                               first=(dc == 0), last=(dc == NDC - 1))
                    vt = vttp.tile([P, S], BF16, tag="vtt", name="vtt")
                    nc.vector.tensor_copy(vt[:], psv[:])
                    # transpose vT chunk into row-major v
                    for g in range(2):
                        pt = psp.tile([P, 512], BF16, tag="mm", name="mm")
                        for k4 in range(4):
                            sc = g * 4 + k4
                            nc.tensor.transpose(
                                pt[:, k4 * P:(k4 + 1) * P],
                                vt[:, sc * P:(sc + 1) * P], ident[:])
                        for k4 in range(4):
                            sc = g * 4 + k4
                            nc.vector.tensor_copy(
                                vrow[sc][:, oc * P:(oc + 1) * P],
                                pt[:, k4 * P:(k4 + 1) * P])

                # ===== attention =====
                for h in range(H):
                    ocn, rsub = h // 2, 64 * (h % 2)
                    if h % 2 == 0:
                        rept = repp.tile([P, S], BF16, tag="rep", name="rep")
                        nc.sync.dma_start(rept[:], repd.ap()[li, ocn])
                    rbt = rbp.tile([P, S], BF16, tag="rb", name="rb")
                    nc.sync.dma_start(
                        rbt[:], _ap(rbd.ap(), (li * H + h) * S,
                                    [[0, P], [1, S]]))
                    at_tiles = []
                    for qc in range(NQC):
                        psc = psp.tile([P, S], F32, tag="score", name="score")
                        psb = psp.tile([P, S], F32, tag="score", name="score")
                        for jh in range(2):
                            sl = slice(jh * 512, (jh + 1) * 512)
                            mm(psc[:, sl],
                               rq[ocn][rsub:rsub + 64, qc * P:(qc + 1) * P],
                               kT[ocn][rsub:rsub + 64, sl])
                            mm(psb[:, sl],
                               rq[ocn][rsub:rsub + 64, qc * P:(qc + 1) * P],
                               rept[rsub:rsub + 64, sl])
                        braw = brawp.tile([P, S], BF16, tag="braw", name="braw")
                        nc.vector.tensor_tensor(braw[:], psb[:], rbt[:],
                                                op=ALU.add)
                        base = h * NQ * SKW
                        nc.sync.dma_start(
                            _ap(skap, base + qc * P * SKW,
                                [[SKW, P], [1, 1024]]),
                            braw[:])
                        bdt = bdp.tile([P, S], BF16, tag="bd", name="bd")
                        nc.sync.dma_start(
                            bdt[:],
                            _ap(skap, base + qc * P * 2047 + 1023,
                                [[2047, P], [1, 1024]]))
                        nc.vector.tensor_tensor(psc[:], psc[:], bdt[:],
                                                op=ALU.add)
                        at = attnp.tile([P, S], BF16, tag="attn", name="attn")
                        zt = smallp.tile([P, 1], F32, tag="z", name="z")
                        nc.scalar.activation(at[:], psc[:], AF.Exp,
                                             bias=zb_t[:], scale=SCALE,
                                             accum_out=zt[:])
                        zr = smallp.tile([P, 1], F32, tag="zr", name="zr")
                        nc.vector.reciprocal(zr[:], zt[:])
                        nc.vector.tensor_scalar_mul(at[:], at[:], zr[:])
                        at_tiles.append(at)
                    pav = psp.tile([64, NQ], F32, tag="av", name="av")
                    for jc in range(NDC):
                        pt = psp.tile([P, 512], BF16, tag="mm", name="mm")
                        for qc in range(NQC):
                            nc.tensor.transpose(
                                pt[:, qc * P:(qc + 1) * P],
                                at_tiles[qc][:, jc * P:(jc + 1) * P],
                                ident[:])
                        atT = attnTp.tile([P, NQ], BF16, tag="atT", name="atT")
                        nc.vector.tensor_copy(atT[:], pt[:])
                        mm(pav[:], vrow[jc][:, h * DH:(h + 1) * DH], atT[:],
                           first=(jc == 0), last=(jc == NDC - 1))
                    tmpo = vttp.tile([64, NQ], BF16, tag="tmpo", name="tmpo")
                    nc.vector.tensor_copy(tmpo[:], pav[:])
                    nc.sync.dma_start(aoT[ocn][rsub:rsub + 64, :], tmpo[:])

                # ===== Wo + residual + LN1 + xT1 =====
                wo_t = []
                for dc in range(NDC):
                    w = wrhsp.tile([P, D], BF16, tag="wrhs", name="wrhs")
                    nc.sync.dma_start(
                        w[:], wot.ap()[li, dc * P:(dc + 1) * P, :])
                    wo_t.append(w)
                for qc in range(NQC):
                    for o2 in range(2):
                        sl = slice(o2 * 512, (o2 + 1) * 512)
                        pp = psp.tile([P, 512], F32, tag="mm", name="mm")
                        for dc in range(NDC):
                            mm(pp[:], aoT[dc][:, qc * P:(qc + 1) * P],
                               wo_t[dc][:, sl],
                               first=(dc == 0), last=(dc == NDC - 1))
                        nc.vector.tensor_add(x_row[qc][:, sl],
                                             x_row[qc][:, sl], pp[:])
                    layernorm(x_row[qc])
                    xb = xwp.tile([P, D], BF16, tag="xb", name="xb")
                    nc.vector.tensor_copy(xb[:], x_row[qc][:])
                    transpose_to(xT1, xb, qc)

                # ===== FFN =====
                for fh in range(2):
                    gts = []
                    for fc16 in range(16):
                        fc = fh * 16 + fc16
                        w1t_ = wTp.tile([P, D], BF16, tag="w1", name="w1")
                        nc.sync.dma_start(w1t_[:], w1r.ap()[li, fc])
                        ph = psp.tile([P, NQ], F32, tag="mm", name="mm")
                        for dc in range(NDC):
                            mm(ph[:], w1t_[:, dc * P:(dc + 1) * P],
                               xT1[dc][:],
                               first=(dc == 0), last=(dc == NDC - 1))
                        gt = gtp.tile([P, NQ], BF16, tag="gt", name="gt")
                        nc.scalar.activation(gt[:], ph[:], AF.Gelu,
                                             bias=b1_t[:, fc:fc + 1],
                                             scale=1.0)
                        gts.append(gt)
                    for fcg in range(2):
                        w2_t = []
                        for f8 in range(8):
                            w = wrhsp.tile([P, D], BF16, tag="wrhs",
                                           name="wrhs")
                            fc = fh * 16 + fcg * 8 + f8
                            nc.sync.dma_start(
                                w[:], w2t.ap()[li, fc * P:(fc + 1) * P, :])
                            w2_t.append(w)
                        for qc in range(NQC):
                            for o2 in range(2):
                                sl = slice(o2 * 512, (o2 + 1) * 512)
                                pf = psp.tile([P, 512], F32, tag="mm",
                                              name="mm")
                                for f8 in range(8):
                                    mm(pf[:],
                                       gts[fcg * 8 + f8][:,
                                                         qc * P:(qc + 1) * P],
                                       w2_t[f8][:, sl],
                                       first=(f8 == 0), last=(f8 == 7))
                                if fh == 0 and fcg == 0:
                                    nc.vector.tensor_add(ff2acc[qc][:, sl],
                                                         pf[:], b2_t[:, sl])
                                else:
                                    nc.vector.tensor_add(ff2acc[qc][:, sl],
                                                         ff2acc[qc][:, sl],
                                                         pf[:])

                # ===== residual + LN2; next-layer prep or output =====
                for qc in range(NQC):
                    nc.vector.tensor_add(x_row[qc][:], x_row[qc][:],
                                         ff2acc[qc][:])
                    layernorm(x_row[qc])
                    if li == NL - 1:
                        nc.sync.dma_start(
                            yout.ap()[qc * P:(qc + 1) * P, :], x_row[qc][:])
                    else:
                        xb = xwp.tile([P, D], BF16, tag="xb", name="xb")
                        nc.vector.tensor_copy(xb[:], x_row[qc][:])
                        transpose_to(xTown, xb, qc)
                if li < NL - 1:
                    for dc in range(NDC):
                        nc.sync.dma_start(
                            _ap(agin.ap(), dc * P * NQ, [[NQ, P], [1, NQ]]),
                            xTown[dc][:])

    nc.finalize()
    return nc


def _prep_host(inputs):
    bf = ml_dtypes.bfloat16
    embed = np.asarray(inputs["embed"], np.float32)
    seq = np.asarray(inputs["seq"]).astype(np.int64)
    x0 = embed[seq]                                   # [B, S, D] f32

    Wq = np.asarray(inputs["Wq"], np.float32)
    Wk = np.asarray(inputs["Wk"], np.float32)
    Wv = np.asarray(inputs["Wv"], np.float32)
    Wo = np.asarray(inputs["Wo"], np.float32)
    w1 = np.asarray(inputs["w1"], np.float32)
    w2 = np.asarray(inputs["w2"], np.float32)
    b1 = np.asarray(inputs["b1"], np.float32)
    b2 = np.asarray(inputs["b2"], np.float32)
    r_emb = np.asarray(inputs["r_emb"], np.float32)
    r_w_bias = np.asarray(inputs["r_w_bias"], np.float32)
    r_bias = np.asarray(inputs["r_bias"], np.float32)

    def packl(WT):   # [D, D] -> [NDC, P, D] lhsT pack
        return np.ascontiguousarray(
            WT.reshape(NDC, P, NDC, P).transpose(2, 1, 0, 3)
            .reshape(NDC, P, D))

    wproj = np.stack([
        np.stack([packl(Wq[l].T), packl(Wk[l].T), packl(Wv[l].T)])
        for l in range(NL)]).astype(bf)
    wot = np.stack([Wo[l].T for l in range(NL)]).astype(bf)
    w1r = np.stack([
        np.ascontiguousarray(
            w1[l].T.reshape(NDC, P, NFC, P).transpose(2, 1, 0, 3)
            .reshape(NFC, P, D))
        for l in range(NL)]).astype(bf)
    w2t = np.stack([w2[l].T for l in range(NL)]).astype(bf)
    b1r = np.stack([b1[l].reshape(NFC, P).T for l in range(NL)])
    b1r = np.ascontiguousarray(b1r).astype(np.float32)
    b2r = b2.astype(np.float32)
    rwbr = np.stack([r_w_bias[l].reshape(D).reshape(NDC, P).T
                     for l in range(NL)])
    rwbr = np.ascontiguousarray(rwbr).astype(np.float32)

    # rep: per head-pair stacked re^T; rb_adj = rb - rwb @ re^T separately
    off = r_emb.shape[2] - S     # MAX_KLEN - S
    rep = np.empty((NL, NDC, P, S), np.float32)
    rba = np.empty((NL, H, S), np.float32)
    for l in range(NL):
        for h in range(H):
            re = r_emb[l, h, off:, :]            # [S, DH]
            rep[l, h // 2, (h % 2) * 64:(h % 2) * 64 + 64] = re.T
            rba[l, h] = r_bias[l, h, off:] - r_w_bias[l, h] @ re.T

    ident = np.eye(P, dtype=bf)

    in_maps = []
    for c in range(NCORES):
        b, half = c // 2, c % 2
        q0 = half * NQ
        xr = np.ascontiguousarray(x0[b, q0:q0 + NQ]).astype(np.float32)
        xt = np.ascontiguousarray(x0[b, q0:q0 + NQ].T).astype(bf)
        repc = np.ascontiguousarray(np.roll(rep, q0, axis=-1)).astype(bf) \
            if q0 else rep.astype(bf)
        rbac = np.ascontiguousarray(np.roll(rba, q0, axis=-1)).astype(bf) \
            if q0 else rba.astype(bf)
        pvec = np.arange(P, dtype=np.int32)
        agix = np.empty((P, 2 * NDC), np.int32)
        for dc in range(NDC):
            for hh in range(2):
                agix[:, dc * 2 + hh] = (2 * b + hh) * D + dc * P + pvec
        in_maps.append({
            "xrow0": xr, "xt0": xt, "wproj": wproj, "wot": wot,
            "w1r": w1r, "w2t": w2t, "b1r": b1r, "b2r": b2r,
            "repd": repc, "rbd": rbac, "rwbr": rwbr, "ident": ident,
            "agidx": agix,
        })
    return in_maps


def run(inputs, trace=False):
    if "nc" not in _cache:
        _cache["nc"] = build()
    nc = _cache["nc"]
    in_maps = _prep_host(inputs)
    res = run_bass_kernel_spmd(nc, in_maps, list(range(NCORES)),
                               trace=trace)
    y = np.zeros((B, S, D), np.float32)
    for c in range(NCORES):
        b, half = c // 2, c % 2
        y[b, half * NQ:(half + 1) * NQ] = res.results[c]["y"]
    return y, res


def kernel(**inputs) -> np.ndarray:
    y, _ = run(inputs)
    return y


def timed_run(inputs, iters=3):
    """Correctness + device-exec timing: replicate run_bass_via_pjrt's
    multi-core path with inputs pre-staged on device."""
    import time
    import jax
    import jax.numpy as jnp
    from jax.sharding import Mesh, PartitionSpec
    from jax.experimental.shard_map import shard_map
    from concourse import bass2jax, mybir as _mb

    if "nc" not in _cache:
        _cache["nc"] = build()
    nc = _cache["nc"]
    in_maps = _prep_host(inputs)
    bass2jax.install_neuronx_cc_hook()

    partition_name = (nc.partition_id_tensor.name
                      if nc.partition_id_tensor else None)
    in_names, out_names, out_avals, zero_outs = [], [], [], []
    for alloc in nc.m.functions[0].allocations:
        if not isinstance(alloc, _mb.MemoryLocationSet):
            continue
        name = alloc.memorylocations[0].name
        if alloc.kind == "ExternalInput":
            if name != partition_name:
                in_names.append(name)
        elif alloc.kind == "ExternalOutput":
            out_names.append(name)
            shape = tuple(alloc.tensor_shape)
            dtype = _mb.dt.np(alloc.dtype)
            out_avals.append(jax.core.ShapedArray(shape, dtype))
            zero_outs.append(np.zeros(shape, dtype))
    n_params = len(in_names)
    n_outs = len(out_avals)
    all_in = list(in_names) + list(out_names)
    if partition_name is not None:
        all_in.append(partition_name)

    def _body(*args):
        operands = list(args)
        if partition_name is not None:
            operands.append(bass2jax.partition_id_tensor())
        outs = bass2jax._bass_exec_p.bind(
            *operands, out_avals=tuple(out_avals),
            in_names=tuple(all_in[:n_params] + out_names),
            out_names=tuple(out_names),
            lowering_input_output_aliases=(), sim_require_finite=True,
            sim_require_nnan=True, nc=nc)
        return tuple(outs)

    devices = jax.devices()[:NCORES]
    mesh = Mesh(np.asarray(devices), ("core",))
    in_specs = (PartitionSpec("core"),) * (n_params + n_outs)
    out_specs = (PartitionSpec("core"),) * n_outs
    fn = jax.jit(shard_map(_body, mesh=mesh, in_specs=in_specs,
                           out_specs=out_specs, check_rep=False),
                 keep_unused=True)
    concat_in = [np.concatenate([np.asarray(in_maps[c][nm])
                                 for c in range(NCORES)], axis=0)
                 for nm in in_names]
    concat_zeros = [np.zeros((NCORES * z.shape[0], *z.shape[1:]), z.dtype)
                    for z in zero_outs]
    staged = [jax.device_put(a) for a in concat_in + concat_zeros]
    out = fn(*staged)
    jax.block_until_ready(out)
    times = []
    for _ in range(iters):
        t0 = time.perf_counter()
        out = fn(*staged)
        jax.block_until_ready(out)
        times.append(time.perf_counter() - t0)
    y = np.zeros((B, S, D), np.float32)
    arr = np.asarray(out[out_names.index("y")]).reshape(NCORES, NQ, D)
    for c in range(NCORES):
        b_, half = c // 2, c % 2
        y[b_, half * NQ:(half + 1) * NQ] = arr[c]
    return y, min(times)



# revision 8
# speedup vs baseline: 2.0086x; 2.0086x over previous
"""Trainium2 Bass kernel for nn_CompressiveEncoder (4-layer relative-position
transformer encoder, B=4 S=1024 D=1024 H=16 FF=4096).

Sharding: 8 cores = (batch b = c//2) x (query-half q0 = (c%2)*512).
Each layer AllGathers the bf16 transposed hidden state; each core selects its
batch pair via indirect DMA, computes K/V for its full batch and attention +
FFN for its own 512 query rows.

Attention is computed in transposed [k, q] orientation: AC^T comes straight
from kT/rq matmuls, and the Music-Transformer shift term BD is accumulated
into the same PSUM banks with transpose-mode matmuls reading skewed DRAM
tiles (f32).  The skew is restricted to its lower-triangular support
(core-local row index >= key index), matching the baseline semantics.
Softmax normalization is folded into the attention@V matmul via a ones
column appended to V, so exp() outputs are consumed unnormalized and each
head is scaled by 1/Z once on the [64, 512] output.
"""

import sys

sys.path.insert(0, "/opt/trn_rl_repo")

import numpy as np
import ml_dtypes

import concourse.bass as bass
import concourse.mybir as mybir
import concourse.tile as tile
from concourse import bacc
from concourse.bass_utils import run_bass_kernel_spmd

BF16 = mybir.dt.bfloat16
F32 = mybir.dt.float32
AF = mybir.ActivationFunctionType
ALU = mybir.AluOpType

NL, D, H, DH, S, FF_DIM = 4, 1024, 16, 64, 1024, 4096
B = 4
NCORES = 8
NQ = 512           # query rows per core
P = 128
SCALE = float(H) ** -0.5   # reference scales by 1/sqrt(heads) = 0.25
EPS = 1e-5
NQC = NQ // P      # 4
NDC = D // P       # 8
NFC = FF_DIM // P  # 32
SKR = 1152         # skew row width (f32 elements): 1024 data + 128 zero pad

_cache: dict = {}


def _ap(t, off, pattern):
    return bass.AP(tensor=t.tensor, offset=t.offset + off, ap=pattern)


def build():
    nc = bacc.Bacc("TRN2", target_bir_lowering=False, debug=False,
                   num_devices=NCORES)

    xrow0 = nc.dram_tensor("xrow0", [NQ, D], F32, kind="ExternalInput")
    xt0 = nc.dram_tensor("xt0", [D, NQ], BF16, kind="ExternalInput")
    # q/k lhsT packs
    wproj = nc.dram_tensor("wproj", [NL, 2, NDC, P, D], BF16,
                           kind="ExternalInput")
    wvt_d = nc.dram_tensor("wvt", [NL, NDC, P, D], BF16,
                           kind="ExternalInput")     # Wv.T rows (rhs pack)
    wot = nc.dram_tensor("wot", [NL, D, D], BF16, kind="ExternalInput")
    w1r = nc.dram_tensor("w1r", [NL, NFC, P, D], BF16, kind="ExternalInput")
    w2t = nc.dram_tensor("w2t", [NL, FF_DIM, D], BF16, kind="ExternalInput")
    b1r = nc.dram_tensor("b1r", [NL, P, NFC], F32, kind="ExternalInput")
    b2r = nc.dram_tensor("b2r", [NL, D], F32, kind="ExternalInput")
    repd = nc.dram_tensor("repd", [NL, NDC, P, 512], BF16,
                          kind="ExternalInput")      # rolled re^T, last 512 js
    rbd = nc.dram_tensor("rbd", [NL, H, 512], BF16,
                         kind="ExternalInput")       # rolled rb adj, last 512
    rwbr = nc.dram_tensor("rwbr", [NL, P, NDC], F32, kind="ExternalInput")
    ident_d = nc.dram_tensor("ident", [P, P], BF16, kind="ExternalInput")
    identf_d = nc.dram_tensor("identf", [P, P], F32, kind="ExternalInput")
    agidx = nc.dram_tensor("agidx", [P, 2 * NDC], mybir.dt.int32,
                           kind="ExternalInput")
    yout = nc.dram_tensor("y", [NQ, D], F32, kind="ExternalOutput")

    # internal DRAM
    sk = nc.dram_tensor("sk_buf", [H * NQ * SKR], F32)
    zd = nc.dram_tensor("zd_buf", [H * NQ], F32)
    agin = nc.dram_tensor("agin_buf", [D * NQ], BF16)
    agout = nc.dram_tensor("agout_buf", [NCORES * D, NQ], BF16,
                           addr_space="Shared")

    from contextlib import ExitStack
    with tile.TileContext(nc) as tc, ExitStack() as stk:
            singles = stk.enter_context(tc.tile_pool(name="singles", bufs=1))
            wTp = stk.enter_context(tc.tile_pool(name="wT", bufs=3))
            wrhsp = stk.enter_context(tc.tile_pool(name="wrhs", bufs=9))
            gtp = stk.enter_context(tc.tile_pool(name="gt", bufs=17))
            atTp = stk.enter_context(tc.tile_pool(name="atT", bufs=16))
            bdtp = stk.enter_context(tc.tile_pool(name="bdt", bufs=6))
            brawp = stk.enter_context(tc.tile_pool(name="braw", bufs=3))
            repp = stk.enter_context(tc.tile_pool(name="rep", bufs=2))
            rbp = stk.enter_context(tc.tile_pool(name="rb", bufs=2))
            zrowp = stk.enter_context(tc.tile_pool(name="zrow", bufs=2))
            zrtp = stk.enter_context(tc.tile_pool(name="zrt", bufs=2))
            xwp = stk.enter_context(tc.tile_pool(name="xw", bufs=2))
            smallp = stk.enter_context(tc.tile_pool(name="small", bufs=16))
            psp = stk.enter_context(
                tc.tile_pool(name="psum", bufs=1, space="PSUM"))
            # ------------- persistent SBUF state -------------
            x_row = [singles.tile([P, D], F32, tag=f"xrow{i}", name=f"xrow{i}")
                     for i in range(NQC)]
            xTown = [singles.tile([P, NQ], BF16, tag=f"xto{i}", name=f"xto{i}")
                     for i in range(NDC)]       # my own x^T (this layer's in)
            xT = [singles.tile([P, 2 * NQ], BF16, tag=f"xt{i}", name=f"xt{i}")
                  for i in range(NDC)]          # gathered x^T, my full batch
            xT1 = [singles.tile([P, NQ], BF16, tag=f"xt1_{i}", name=f"xt1_{i}")
                   for i in range(NDC)]         # post-LN1 x^T, my rows
            kT = [singles.tile([P, S], BF16, tag=f"kt{i}", name=f"kt{i}")
                  for i in range(NDC)]
            # v rows + per-head ones column (even head: [dh,1], odd: [1,dh])
            vrow = [singles.tile([P, H * 65], BF16, tag=f"vr{i}",
                                 name=f"vr{i}")
                    for i in range(NDC)]
            rq = [singles.tile([P, NQ], BF16, tag=f"rq{i}", name=f"rq{i}")
                  for i in range(NDC)]
            aoT = [singles.tile([P, NQ], BF16, tag=f"aoT{i}", name=f"aoT{i}")
                   for i in range(NDC)]
            ident = singles.tile([P, P], BF16, tag="ident", name="ident")
            identf = singles.tile([P, P], F32, tag="identf", name="identf")
            eps_t = singles.tile([P, 1], F32, tag="eps", name="eps")
            zb_t = singles.tile([P, 1], F32, tag="zbias", name="zbias")
            b2_t = singles.tile([P, D], F32, tag="b2rep", name="b2rep")
            b1_t = singles.tile([P, NFC], F32, tag="b1", name="b1")
            rwb_t = singles.tile([P, NDC], F32, tag="rwb", name="rwb")
            agix_t = singles.tile([P, 2 * NDC], mybir.dt.int32, tag="agix",
                                  name="agix")
            zeros_t = singles.tile([P, P], F32, tag="zeros", name="zeros")

            nc.sync.dma_start(ident[:], ident_d.ap())
            nc.sync.dma_start(identf[:], identf_d.ap())
            nc.sync.dma_start(agix_t[:], agidx.ap())
            nc.vector.memset(eps_t[:], EPS)
            nc.vector.memset(zb_t[:], 0.0)
            nc.vector.memset(zeros_t[:], 0.0)
            # ones column in vrow: head h -> col h*65+64 (for softmax Z)
            for dc in range(NDC):
                vv = vrow[dc].rearrange("p (h c) -> p h c", h=H)
                nc.vector.memset(vv[:, :, 64:65], 1.0)
            skap = sk.ap()
            # zero skew pad cols [1024, 1152) once
            for blk in range(H * NQ // P):
                dst = _ap(skap, blk * P * SKR + 1024, [[SKR, P], [1, P]])
                nc.sync.dma_start(dst, zeros_t[:, :])
            for qc in range(NQC):
                nc.sync.dma_start(x_row[qc][:],
                                  xrow0.ap()[qc * P:(qc + 1) * P, :])
            for dc in range(NDC):
                nc.sync.dma_start(xTown[dc][:],
                                  xt0.ap()[dc * P:(dc + 1) * P, :])
            nc.sync.dma_start(agin.ap(), xt0.ap())

            def mm(out, lhsT, rhs, first=True, last=True):
                nc.tensor.matmul(out, lhsT, rhs, start=first, stop=last)

            def layernorm(xr):
                st = smallp.tile([P, 2, 6], F32, tag="bnst", name="bnst")
                nc.vector.bn_stats(st[:, 0, :], xr[:, 0:512])
                nc.vector.bn_stats(st[:, 1, :], xr[:, 512:1024])
                mv = smallp.tile([P, 2], F32, tag="bnmv", name="bnmv")
                nc.vector.bn_aggr(mv[:], st[:])
                sd = smallp.tile([P, 1], F32, tag="sd", name="sd")
                nc.scalar.activation(sd[:], mv[:, 1:2], AF.Sqrt,
                                     bias=eps_t[:], scale=1.0)
                rs = smallp.tile([P, 1], F32, tag="rs", name="rs")
                nc.vector.reciprocal(rs[:], sd[:])
                nc.vector.tensor_scalar(
                    out=xr[:], in0=xr[:], scalar1=mv[:, 0:1],
                    scalar2=rs[:], op0=ALU.subtract, op1=ALU.mult)

            def transpose_to(dsts, src_bf16, qc):
                """src [128(q), 1024(d)] -> dsts[dc][:, qc*128:+128]."""
                for g in range(2):
                    pt = psp.tile([P, 512], BF16, tag="pt", name="pt",
                                  bufs=2, padded_shape=[P, 1024])
                    for k4 in range(4):
                        dc = g * 4 + k4
                        nc.tensor.transpose(
                            pt[:, k4 * P:(k4 + 1) * P],
                            src_bf16[:, dc * P:(dc + 1) * P], ident[:])
                    for k4 in range(4):
                        dc = g * 4 + k4
                        nc.vector.tensor_copy(
                            dsts[dc][:, qc * P:(qc + 1) * P],
                            pt[:, k4 * P:(k4 + 1) * P])

            for li in range(NL):
                # ===== allgather x^T (async; overlapped by q-proj + shift) ==
                nc.gpsimd.collective_compute(
                    "AllGather", ALU.bypass,
                    replica_groups=[list(range(NCORES))],
                    ins=[agin.ap()], outs=[agout.ap()],
                )

                nc.sync.dma_start(rwb_t[:], rwbr.ap()[li])
                nc.sync.dma_start(b1_t[:], b1r.ap()[li])
                nc.sync.dma_start(
                    b2_t[:], _ap(b2r.ap(), li * D, [[0, P], [1, D]]))

                # ===== q projection (own rows; AG-independent) =====
                for oc in range(NDC):
                    wq = wTp.tile([P, D], BF16, tag="wq", name="wq")
                    nc.sync.dma_start(wq[:], wproj.ap()[li, 0, oc])
                    ps = psp.tile([P, NQ], F32, tag="mm", name="mm", bufs=4)
                    for dc in range(NDC):
                        mm(ps[:], wq[:, dc * P:(dc + 1) * P], xTown[dc][:],
                           first=(dc == 0), last=(dc == NDC - 1))
                    nc.vector.tensor_scalar(
                        out=rq[oc][:], in0=ps[:],
                        scalar1=rwb_t[:, oc:oc + 1],
                        scalar2=None, op0=ALU.add)

                # ===== B~ scores + skew write (AG-independent) =====
                for ocn in range(NDC):
                    rept = repp.tile([P, 512], BF16, tag="rep", name="rep")
                    nc.sync.dma_start(rept[:], repd.ap()[li, ocn])
                    for hh in range(2):
                        h = 2 * ocn + hh
                        rsub = 64 * hh
                        rbt = rbp.tile([P, 512], BF16, tag="rb", name="rb")
                        nc.sync.dma_start(
                            rbt[:], _ap(rbd.ap(), (li * H + h) * 512,
                                        [[0, P], [1, 512]]))
                        for qc in range(NQC):
                            w = (qc + 1) * P
                            psb = psp.tile([P, NQ], F32, tag="mm", name="mm",
                                           bufs=4)
                            mm(psb[:, 0:w],
                               rq[ocn][rsub:rsub + 64, qc * P:(qc + 1) * P],
                               rept[rsub:rsub + 64, 512 - w:512])
                            braw = brawp.tile([P, 512], F32, tag="braw",
                                              name="braw")
                            nc.vector.tensor_tensor(
                                braw[:, 0:w], psb[:, 0:w],
                                rbt[:, 512 - w:512], op=ALU.add)
                            base = h * NQ * SKR + qc * P * SKR + (896 - qc * P)
                            nc.sync.dma_start(
                                _ap(skap, base, [[SKR, P], [1, w]]),
                                braw[:, 0:w])

                # ===== gather my batch pair (waits on AG) =====
                for dc in range(NDC):
                    for half in range(2):
                        nc.gpsimd.indirect_dma_start(
                            out=xT[dc][:, half * NQ:(half + 1) * NQ],
                            out_offset=None,
                            in_=agout.ap(),
                            in_offset=bass.IndirectOffsetOnAxis(
                                ap=agix_t[:, dc * 2 + half:
                                          dc * 2 + half + 1],
                                axis=0,
                            ),
                        )

                # ===== k projection -> kT [dh, k] =====
                for oc in range(NDC):
                    wk = wTp.tile([P, D], BF16, tag="wq", name="wq")
                    nc.sync.dma_start(wk[:], wproj.ap()[li, 1, oc])
                    for jh in range(2):
                        psk = psp.tile([P, 512], F32, tag="mm", name="mm",
                                       bufs=4)
                        for dc in range(NDC):
                            mm(psk[:],
                               wk[:, dc * P:(dc + 1) * P],
                               xT[dc][:, jh * 512:(jh + 1) * 512],
                               first=(dc == 0), last=(dc == NDC - 1))
                        nc.vector.tensor_copy(
                            kT[oc][:, jh * 512:(jh + 1) * 512], psk[:])

                # ===== v projection -> vrow [k, dh] directly =====
                wvts = []
                for dc in range(NDC):
                    w = wrhsp.tile([P, D], BF16, tag="wrhs", name="wvt")
                    nc.sync.dma_start(w[:], wvt_d.ap()[li, dc])
                    wvts.append(w)
                for kc in range(NDC):
                    pv = [psp.tile([P, 512], F32, tag="mm", name="mm", bufs=4)
                          for _ in range(2)]
                    for dc in range(NDC):
                        for half in range(2):
                            mm(pv[half][:],
                               xT[dc][:, kc * P:(kc + 1) * P],
                               wvts[dc][:, half * 512:(half + 1) * 512],
                               first=(dc == 0), last=(dc == NDC - 1))
                    vv = vrow[kc].rearrange("p (h c) -> p h c", h=H)
                    for half in range(2):
                        hbase = half * 8
                        sv = pv[half].rearrange("p (h c) -> p h c", h=8)
                        nc.vector.tensor_copy(
                            vv[:, hbase:hbase + 8, 0:64], sv[:])

                # ===== attention (transposed scores [k, q]) =====
                def emit_scores(h):
                    ocn, rsub = h // 2, 64 * (h % 2)
                    bdts = []
                    for qc in range(NQC):
                        w = (qc + 1) * P
                        bdt = bdtp.tile([P, 512], F32, tag="bdt", name="bdt")
                        base = h * NQ * SKR + qc * P * (SKR - 1) + 1023
                        nc.sync.dma_start(
                            bdt[:, 0:w],
                            _ap(skap, base, [[SKR - 1, P], [1, w]]))
                        bdts.append(bdt)
                    psts = []
                    for jc in range(8):
                        pst = psp.tile([P, 512], F32, tag="mm", name="mm",
                                       bufs=4)
                        mm(pst[:],
                           kT[ocn][rsub:rsub + 64, jc * P:(jc + 1) * P],
                           rq[ocn][rsub:rsub + 64, :],
                           first=True, last=(jc >= NQC))
                        psts.append(pst)
                    ats = []
                    for qc in range(NQC):
                        for jc in range(qc + 1):
                            nc.tensor.matmul(
                                psts[jc][:, qc * P:(qc + 1) * P],
                                bdts[qc][:, jc * P:(jc + 1) * P],
                                identf[:], is_transpose=True,
                                start=False, stop=(qc == NQC - 1))
                    for jc in range(8):
                        at = atTp.tile([P, 512], BF16, tag="atT", name="atT")
                        nc.scalar.activation(at[:], psts[jc][:], AF.Exp,
                                             bias=zb_t[:], scale=SCALE)
                        ats.append(at)
                    return ats

                def emit_av(h, ats):
                    ocn, rsub = h // 2, 64 * (h % 2)
                    # pav rows [0:65]: AV rows 0..63, Z (ones-col sum) row 64
                    pav = psp.tile([P, 512], F32, tag="pav", name="pav",
                                   bufs=2)
                    for jc in range(NDC):
                        mm(pav[0:65, :],
                           vrow[jc][:, h * 65:h * 65 + 65],
                           ats[jc][:],
                           first=(jc == 0), last=(jc == NDC - 1))
                    zrow = zrowp.tile([P, 512], F32, tag="zrow", name="zrow")
                    nc.vector.reciprocal(zrow[64:65, :], pav[64:65, :])
                    nc.sync.dma_start(_ap(zd.ap(), h * NQ, [[1, NQ]]),
                                      zrow[64:65, :])
                    zrt = zrtp.tile([64, 512], F32, tag="zrt", name="zrt")
                    nc.sync.dma_start(
                        zrt[:], _ap(zd.ap(), h * NQ, [[0, 64], [1, NQ]]))
                    tmpo = zrowp.tile([64, 512], BF16, tag="tmpo",
                                      name="tmpo")
                    nc.vector.tensor_tensor(tmpo[:], pav[0:64, :], zrt[:],
                                            op=ALU.mult)
                    nc.sync.dma_start(aoT[ocn][rsub:rsub + 64, :], tmpo[:])

                prev = None
                for h in range(H):
                    ats = emit_scores(h)
                    if prev is not None:
                        emit_av(h - 1, prev)
                    prev = ats
                emit_av(H - 1, prev)

                # ===== Wo + residual + LN1 + xT1 =====
                wo_t = []
                for dc in range(NDC):
                    w = wrhsp.tile([P, D], BF16, tag="wrhs", name="wrhs")
                    nc.sync.dma_start(
                        w[:], wot.ap()[li, dc * P:(dc + 1) * P, :])
                    wo_t.append(w)
                for qc in range(NQC):
                    pp = [psp.tile([P, 512], F32, tag="mm", name="mm", bufs=4)
                          for _ in range(2)]
                    for dc in range(NDC):
                        for o2 in range(2):
                            mm(pp[o2][:], aoT[dc][:, qc * P:(qc + 1) * P],
                               wo_t[dc][:, o2 * 512:(o2 + 1) * 512],
                               first=(dc == 0), last=(dc == NDC - 1))
                    for o2 in range(2):
                        sl = slice(o2 * 512, (o2 + 1) * 512)
                        nc.vector.tensor_add(x_row[qc][:, sl],
                                             x_row[qc][:, sl], pp[o2][:])
                    layernorm(x_row[qc])
                    xb = xwp.tile([P, D], BF16, tag="xb", name="xb")
                    nc.vector.tensor_copy(xb[:], x_row[qc][:])
                    transpose_to(xT1, xb, qc)

                # b2 pre-add into residual stream
                for qc in range(NQC):
                    nc.vector.tensor_add(x_row[qc][:], x_row[qc][:], b2_t[:])

                # ===== FFN =====
                for fh in range(2):
                    gts = []
                    for fc16 in range(16):
                        fc = fh * 16 + fc16
                        w1t_ = wTp.tile([P, D], BF16, tag="wq", name="w1")
                        nc.sync.dma_start(w1t_[:], w1r.ap()[li, fc])
                        ph = psp.tile([P, NQ], F32, tag="mm", name="mm",
                                      bufs=4)
                        for dc in range(NDC):
                            mm(ph[:], w1t_[:, dc * P:(dc + 1) * P],
                               xT1[dc][:],
                               first=(dc == 0), last=(dc == NDC - 1))
                        gt = gtp.tile([P, NQ], BF16, tag="gt", name="gt")
                        nc.scalar.activation(gt[:], ph[:], AF.Gelu,
                                             bias=b1_t[:, fc:fc + 1],
                                             scale=1.0)
                        gts.append(gt)
                    for fcg in range(2):
                        w2_t = []
                        for f8 in range(8):
                            w = wrhsp.tile([P, D], BF16, tag="wrhs",
                                           name="wrhs")
                            fc = fh * 16 + fcg * 8 + f8
                            nc.sync.dma_start(
                                w[:], w2t.ap()[li, fc * P:(fc + 1) * P, :])
                            w2_t.append(w)
                        for qc in range(NQC):
                            for o2 in range(2):
                                sl = slice(o2 * 512, (o2 + 1) * 512)
                                pf = psp.tile([P, 512], F32, tag="mm",
                                              name="mm", bufs=4)
                                for f8 in range(8):
                                    mm(pf[:],
                                       gts[fcg * 8 + f8][:,
                                                         qc * P:(qc + 1) * P],
                                       w2_t[f8][:, sl],
                                       first=(f8 == 0), last=(f8 == 7))
                                nc.vector.tensor_add(x_row[qc][:, sl],
                                                     x_row[qc][:, sl],
                                                     pf[:])

                # ===== LN2; next-layer prep or output =====
                for qc in range(NQC):
                    layernorm(x_row[qc])
                    if li == NL - 1:
                        nc.sync.dma_start(
                            yout.ap()[qc * P:(qc + 1) * P, :], x_row[qc][:])
                    else:
                        xb = xwp.tile([P, D], BF16, tag="xb", name="xb")
                        nc.vector.tensor_copy(xb[:], x_row[qc][:])
                        transpose_to(xTown, xb, qc)
                if li < NL - 1:
                    for dc in range(NDC):
                        nc.sync.dma_start(
                            _ap(agin.ap(), dc * P * NQ, [[NQ, P], [1, NQ]]),
                            xTown[dc][:])

    nc.finalize()
    return nc


def _prep_host(inputs):
    bf = ml_dtypes.bfloat16
    embed = np.asarray(inputs["embed"], np.float32)
    seq = np.asarray(inputs["seq"]).astype(np.int64)
    x0 = embed[seq]                                   # [B, S, D] f32

    Wq = np.asarray(inputs["Wq"], np.float32)
    Wk = np.asarray(inputs["Wk"], np.float32)
    Wv = np.asarray(inputs["Wv"], np.float32)
    Wo = np.asarray(inputs["Wo"], np.float32)
    w1 = np.asarray(inputs["w1"], np.float32)
    w2 = np.asarray(inputs["w2"], np.float32)
    b1 = np.asarray(inputs["b1"], np.float32)
    b2 = np.asarray(inputs["b2"], np.float32)
    r_emb = np.asarray(inputs["r_emb"], np.float32)
    r_w_bias = np.asarray(inputs["r_w_bias"], np.float32)
    r_bias = np.asarray(inputs["r_bias"], np.float32)

    def packl(WT):   # [D, D] -> [NDC, P, D] lhsT pack
        return np.ascontiguousarray(
            WT.reshape(NDC, P, NDC, P).transpose(2, 1, 0, 3)
            .reshape(NDC, P, D))

    wproj = np.stack([
        np.stack([packl(Wq[l].T), packl(Wk[l].T)])
        for l in range(NL)]).astype(bf)
    wvt = np.stack([Wv[l].T.reshape(NDC, P, D) for l in range(NL)]).astype(bf)
    wot = np.stack([Wo[l].T for l in range(NL)]).astype(bf)
    w1r = np.stack([
        np.ascontiguousarray(
            w1[l].T.reshape(NDC, P, NFC, P).transpose(2, 1, 0, 3)
            .reshape(NFC, P, D))
        for l in range(NL)]).astype(bf)
    w2t = np.stack([w2[l].T for l in range(NL)]).astype(bf)
    b1r = np.stack([b1[l].reshape(NFC, P).T for l in range(NL)])
    b1r = np.ascontiguousarray(b1r).astype(np.float32)
    b2r = b2.astype(np.float32)
    rwbr = np.stack([r_w_bias[l].reshape(D).reshape(NDC, P).T
                     for l in range(NL)])
    rwbr = np.ascontiguousarray(rwbr).astype(np.float32)

    # rep: per head-pair stacked re^T; rb_adj = rb - rwb @ re^T separately
    off = r_emb.shape[2] - S     # MAX_KLEN - S
    rep = np.empty((NL, NDC, P, S), np.float32)
    rba = np.empty((NL, H, S), np.float32)
    for l in range(NL):
        for h in range(H):
            re = r_emb[l, h, off:, :]            # [S, DH]
            rep[l, h // 2, (h % 2) * 64:(h % 2) * 64 + 64] = re.T
            rba[l, h] = r_bias[l, h, off:] - r_w_bias[l, h] @ re.T

    ident = np.eye(P, dtype=bf)
    identf = np.eye(P, dtype=np.float32)

    in_maps = []
    for c in range(NCORES):
        b, half = c // 2, c % 2
        q0 = half * NQ
        xr = np.ascontiguousarray(x0[b, q0:q0 + NQ]).astype(np.float32)
        xt = np.ascontiguousarray(x0[b, q0:q0 + NQ].T).astype(bf)
        repc = np.roll(rep, q0, axis=-1) if q0 else rep
        rbac = np.roll(rba, q0, axis=-1) if q0 else rba
        repc = np.ascontiguousarray(repc[..., 512:]).astype(bf)
        rbac = np.ascontiguousarray(rbac[..., 512:]).astype(bf)
        pvec = np.arange(P, dtype=np.int32)
        agix = np.empty((P, 2 * NDC), np.int32)
        for dc in range(NDC):
            for hh in range(2):
                agix[:, dc * 2 + hh] = (2 * b + hh) * D + dc * P + pvec
        in_maps.append({
            "xrow0": xr, "xt0": xt, "wproj": wproj, "wvt": wvt, "wot": wot,
            "w1r": w1r, "w2t": w2t, "b1r": b1r, "b2r": b2r,
            "repd": repc, "rbd": rbac, "rwbr": rwbr, "ident": ident,
            "identf": identf, "agidx": agix,
        })
    return in_maps


def run(inputs, trace=False):
    if "nc" not in _cache:
        _cache["nc"] = build()
    nc = _cache["nc"]
    in_maps = _prep_host(inputs)
    res = run_bass_kernel_spmd(nc, in_maps, list(range(NCORES)),
                               trace=trace)
    y = np.zeros((B, S, D), np.float32)
    for c in range(NCORES):
        b, half = c // 2, c % 2
        y[b, half * NQ:(half + 1) * NQ] = res.results[c]["y"]
    return y, res


def kernel(**inputs) -> np.ndarray:
    y, _ = run(inputs)
    return y


# revision 12
# speedup vs baseline: 2.2133x; 1.1019x over previous
"""Trainium2 Bass kernel for nn_CompressiveEncoder (4-layer relative-position
transformer encoder, B=4 S=1024 D=1024 H=16 FF=4096).

Sharding: 8 cores = (batch b = c//2) x (query-half q0 = (c%2)*512).
Each layer AllGathers the bf16 transposed hidden state; each core selects its
batch pair via indirect DMA, computes K/V for its full batch and attention +
FFN for its own 512 query rows.

Attention is computed in transposed [k, q] orientation: AC^T comes straight
from kT/rq matmuls, and the Music-Transformer shift term BD is accumulated
into the same PSUM banks with transpose-mode matmuls reading skewed DRAM
tiles (f32).  The skew is restricted to its lower-triangular support
(core-local row index >= key index), matching the baseline semantics.
Softmax normalization is folded into the attention@V matmul via a ones
column appended to V, so exp() outputs are consumed unnormalized and each
head is scaled by 1/Z once on the [64, 512] output.
"""

import sys

sys.path.insert(0, "/opt/trn_rl_repo")

import numpy as np
import ml_dtypes

import concourse.bass as bass
import concourse.mybir as mybir
import concourse.tile as tile
from concourse import bacc
from concourse.bass_utils import run_bass_kernel_spmd

BF16 = mybir.dt.bfloat16
F32 = mybir.dt.float32
FP8 = mybir.dt.float8e3
AF = mybir.ActivationFunctionType
ALU = mybir.AluOpType

NL, D, H, DH, S, FF_DIM = 4, 1024, 16, 64, 1024, 4096
B = 4
NCORES = 8
NQ = 512           # query rows per core
P = 128
SCALE = float(H) ** -0.5   # reference scales by 1/sqrt(heads) = 0.25
EPS = 1e-5
NQC = NQ // P      # 4
NDC = D // P       # 8
NFC = FF_DIM // P  # 32
SKR = 1152         # skew row width (f32 elements): 1024 data + 128 zero pad
WSC = 64.0         # fp8 e3m4 weight pre-scale for q/k/v/o projections
WSCI = 1.0 / WSC

_cache: dict = {}


def _ap(t, off, pattern):
    return bass.AP(tensor=t.tensor, offset=t.offset + off, ap=pattern)


def build():
    nc = bacc.Bacc("TRN2", target_bir_lowering=False, debug=False,
                   num_devices=NCORES)

    xrow0 = nc.dram_tensor("xrow0", [NQ, D], F32, kind="ExternalInput")
    xt0 = nc.dram_tensor("xt0", [D, NQ], BF16, kind="ExternalInput")
    # q/k lhsT packs
    wproj = nc.dram_tensor("wproj", [NL, 2, NDC, P, D], FP8,
                           kind="ExternalInput")
    wvt_d = nc.dram_tensor("wvt", [NL, NDC, P, D], FP8,
                           kind="ExternalInput")     # Wv.T rows (rhs pack)
    wot = nc.dram_tensor("wot", [NL, D, D], FP8, kind="ExternalInput")
    w1r = nc.dram_tensor("w1r", [NL, NFC, P, D], BF16, kind="ExternalInput")
    w2t = nc.dram_tensor("w2t", [NL, FF_DIM, D], BF16, kind="ExternalInput")
    b1r = nc.dram_tensor("b1r", [NL, P, NFC], F32, kind="ExternalInput")
    b2r = nc.dram_tensor("b2r", [NL, D], F32, kind="ExternalInput")
    repd = nc.dram_tensor("repd", [NL, NDC, P, 512], BF16,
                          kind="ExternalInput")      # rolled re^T, last 512 js
    rbd = nc.dram_tensor("rbd", [NL, H, 512], BF16,
                         kind="ExternalInput")       # rolled rb adj, last 512
    rwbr = nc.dram_tensor("rwbr", [NL, P, NDC], F32, kind="ExternalInput")
    ident_d = nc.dram_tensor("ident", [P, P], BF16, kind="ExternalInput")
    identf_d = nc.dram_tensor("identf", [P, P], F32, kind="ExternalInput")
    yout = nc.dram_tensor("y", [NQ, D], F32, kind="ExternalOutput")

    # internal DRAM
    sk = nc.dram_tensor("sk_buf", [H * NQ * SKR], F32)
    zd = nc.dram_tensor("zd_buf", [H * NQ], F32)
    agin = nc.dram_tensor("agin_buf", [D * NQ], BF16)
    agout = nc.dram_tensor("agout_buf", [2 * D, NQ], BF16)

    from contextlib import ExitStack
    with tile.TileContext(nc) as tc, ExitStack() as stk:
            singles = stk.enter_context(tc.tile_pool(name="singles", bufs=1))
            wTp = stk.enter_context(tc.tile_pool(name="wT", bufs=3))
            wrhsp = stk.enter_context(tc.tile_pool(name="wrhs", bufs=9))
            wop = stk.enter_context(tc.tile_pool(name="wo", bufs=9))
            gtp = stk.enter_context(tc.tile_pool(name="gt", bufs=17))
            atTp = stk.enter_context(tc.tile_pool(name="atT", bufs=14))
            bdtp = stk.enter_context(tc.tile_pool(name="bdt", bufs=5))
            brawp = stk.enter_context(tc.tile_pool(name="braw", bufs=2))
            repp = stk.enter_context(tc.tile_pool(name="rep", bufs=2))
            rbp = stk.enter_context(tc.tile_pool(name="rb", bufs=2))
            zrowp = stk.enter_context(tc.tile_pool(name="zrow", bufs=2))
            zrtp = stk.enter_context(tc.tile_pool(name="zrt", bufs=2))
            xwp = stk.enter_context(tc.tile_pool(name="xw", bufs=2))
            smallp = stk.enter_context(tc.tile_pool(name="small", bufs=16))
            psp = stk.enter_context(
                tc.tile_pool(name="psum", bufs=1, space="PSUM"))
            # ------------- persistent SBUF state -------------
            x_row = [singles.tile([P, D], F32, tag=f"xrow{i}", name=f"xrow{i}")
                     for i in range(NQC)]
            xTown = [singles.tile([P, NQ], BF16, tag=f"xto{i}", name=f"xto{i}")
                     for i in range(NDC)]       # my own x^T (this layer's in)
            xT = [singles.tile([P, 2 * NQ], BF16, tag=f"xt{i}", name=f"xt{i}")
                  for i in range(NDC)]          # gathered x^T, my full batch
            xT1 = [singles.tile([P, NQ], BF16, tag=f"xt1_{i}", name=f"xt1_{i}")
                   for i in range(NDC)]         # post-LN1 x^T, my rows
            kT = [singles.tile([P, S], BF16, tag=f"kt{i}", name=f"kt{i}")
                  for i in range(NDC)]
            # v rows + per-head ones column (even head: [dh,1], odd: [1,dh])
            vrow = [singles.tile([P, H * 65], BF16, tag=f"vr{i}",
                                 name=f"vr{i}")
                    for i in range(NDC)]
            rq = [singles.tile([P, NQ], BF16, tag=f"rq{i}", name=f"rq{i}")
                  for i in range(NDC)]
            aoT = [singles.tile([P, NQ], BF16, tag=f"aoT{i}", name=f"aoT{i}")
                   for i in range(NDC)]
            ident = singles.tile([P, P], BF16, tag="ident", name="ident")
            identf = singles.tile([P, P], F32, tag="identf", name="identf")
            eps_t = singles.tile([P, 1], F32, tag="eps", name="eps")
            zb_t = singles.tile([P, 1], F32, tag="zbias", name="zbias")
            b2_t = singles.tile([P, D], F32, tag="b2rep", name="b2rep")
            b1_t = singles.tile([P, NFC], F32, tag="b1", name="b1")
            rwb_t = singles.tile([P, NDC], F32, tag="rwb", name="rwb")
            zeros_t = singles.tile([P, P], F32, tag="zeros", name="zeros")

            nc.sync.dma_start(ident[:], ident_d.ap())
            nc.sync.dma_start(identf[:], identf_d.ap())
            nc.vector.memset(eps_t[:], EPS)
            nc.vector.memset(zb_t[:], 0.0)
            nc.vector.memset(zeros_t[:], 0.0)
            # ones column in vrow: head h -> col h*65+64 (for softmax Z)
            for dc in range(NDC):
                vv = vrow[dc].rearrange("p (h c) -> p h c", h=H)
                nc.vector.memset(vv[:, :, 64:65], WSC)
            skap = sk.ap()
            # zero skew pad cols [1024, 1152) once
            for blk in range(H * NQ // P):
                dst = _ap(skap, blk * P * SKR + 1024, [[SKR, P], [1, P]])
                nc.sync.dma_start(dst, zeros_t[:, :])
            for qc in range(NQC):
                nc.sync.dma_start(x_row[qc][:],
                                  xrow0.ap()[qc * P:(qc + 1) * P, :])
            for dc in range(NDC):
                nc.sync.dma_start(xTown[dc][:],
                                  xt0.ap()[dc * P:(dc + 1) * P, :])
            nc.sync.dma_start(agin.ap(), xt0.ap())

            def mm(out, lhsT, rhs, first=True, last=True):
                nc.tensor.matmul(out, lhsT, rhs, start=first, stop=last)

            def layernorm(xr):
                st = smallp.tile([P, 2, 6], F32, tag="bnst", name="bnst")
                nc.vector.bn_stats(st[:, 0, :], xr[:, 0:512])
                nc.vector.bn_stats(st[:, 1, :], xr[:, 512:1024])
                mv = smallp.tile([P, 2], F32, tag="bnmv", name="bnmv")
                nc.vector.bn_aggr(mv[:], st[:])
                sd = smallp.tile([P, 1], F32, tag="sd", name="sd")
                nc.scalar.activation(sd[:], mv[:, 1:2], AF.Sqrt,
                                     bias=eps_t[:], scale=1.0)
                rs = smallp.tile([P, 1], F32, tag="rs", name="rs")
                nc.vector.reciprocal(rs[:], sd[:])
                nc.vector.tensor_scalar(
                    out=xr[:], in0=xr[:], scalar1=mv[:, 0:1],
                    scalar2=rs[:], op0=ALU.subtract, op1=ALU.mult)

            def transpose_to(dsts, src_bf16, qc):
                """src [128(q), 1024(d)] -> dsts[dc][:, qc*128:+128]."""
                for g in range(2):
                    pt = psp.tile([P, 512], BF16, tag="pt", name="pt",
                                  bufs=2, padded_shape=[P, 1024])
                    for k4 in range(4):
                        dc = g * 4 + k4
                        nc.tensor.transpose(
                            pt[:, k4 * P:(k4 + 1) * P],
                            src_bf16[:, dc * P:(dc + 1) * P], ident[:])
                    for k4 in range(4):
                        dc = g * 4 + k4
                        nc.vector.tensor_copy(
                            dsts[dc][:, qc * P:(qc + 1) * P],
                            pt[:, k4 * P:(k4 + 1) * P])

            for li in range(NL):
                # ===== allgather x^T (async; overlapped by q-proj + shift) ==
                nc.gpsimd.collective_compute(
                    "AllGather", ALU.bypass,
                    replica_groups=[[2 * g, 2 * g + 1]
                                    for g in range(NCORES // 2)],
                    ins=[agin.ap()], outs=[agout.ap()],
                )

                nc.sync.dma_start(rwb_t[:], rwbr.ap()[li])
                nc.sync.dma_start(b1_t[:], b1r.ap()[li])
                nc.sync.dma_start(
                    b2_t[:], _ap(b2r.ap(), li * D, [[0, P], [1, D]]))

                # ===== q projection (own rows; AG-independent) =====
                for oc in range(NDC):
                    wq = wTp.tile([P, D], FP8, tag="wq", name="wq")
                    nc.sync.dma_start(wq[:], wproj.ap()[li, 0, oc])
                    ps = psp.tile([P, NQ], F32, tag="mm", name="mm", bufs=4)
                    for dc in range(NDC):
                        mm(ps[:], wq[:, dc * P:(dc + 1) * P], xTown[dc][:],
                           first=(dc == 0), last=(dc == NDC - 1))
                    nc.vector.tensor_scalar(
                        out=rq[oc][:], in0=ps[:],
                        scalar1=WSCI, scalar2=rwb_t[:, oc:oc + 1],
                        op0=ALU.mult, op1=ALU.add)

                # ===== B~ scores + skew write (AG-independent) =====
                for ocn in range(NDC):
                    rept = repp.tile([P, 512], BF16, tag="rep", name="rep")
                    nc.sync.dma_start(rept[:], repd.ap()[li, ocn])
                    for hh in range(2):
                        h = 2 * ocn + hh
                        rsub = 64 * hh
                        rbt = rbp.tile([P, 512], BF16, tag="rb", name="rb")
                        nc.sync.dma_start(
                            rbt[:], _ap(rbd.ap(), (li * H + h) * 512,
                                        [[0, P], [1, 512]]))
                        for qc in range(NQC):
                            w = (qc + 1) * P
                            psb = psp.tile([P, NQ], F32, tag="mm", name="mm",
                                           bufs=4)
                            mm(psb[:, 0:w],
                               rq[ocn][rsub:rsub + 64, qc * P:(qc + 1) * P],
                               rept[rsub:rsub + 64, 512 - w:512])
                            braw = brawp.tile([P, 512], F32, tag="braw",
                                              name="braw")
                            nc.vector.tensor_tensor(
                                braw[:, 0:w], psb[:, 0:w],
                                rbt[:, 512 - w:512], op=ALU.add)
                            base = h * NQ * SKR + qc * P * SKR + (896 - qc * P)
                            nc.sync.dma_start(
                                _ap(skap, base, [[SKR, P], [1, w]]),
                                braw[:, 0:w])

                # ===== gather my batch pair (waits on AG) =====
                for dc in range(NDC):
                    for half in range(2):
                        nc.sync.dma_start(
                            xT[dc][:, half * NQ:(half + 1) * NQ],
                            agout.ap()[half * D + dc * P:
                                       half * D + (dc + 1) * P, :])

                # ===== k projection -> kT [dh, k] =====
                for oc in range(NDC):
                    wk = wTp.tile([P, D], FP8, tag="wq", name="wq")
                    nc.sync.dma_start(wk[:], wproj.ap()[li, 1, oc])
                    for jh in range(2):
                        psk = psp.tile([P, 512], F32, tag="mm", name="mm",
                                       bufs=4)
                        for dc in range(NDC):
                            mm(psk[:],
                               wk[:, dc * P:(dc + 1) * P],
                               xT[dc][:, jh * 512:(jh + 1) * 512],
                               first=(dc == 0), last=(dc == NDC - 1))
                        nc.vector.tensor_scalar(
                            out=kT[oc][:, jh * 512:(jh + 1) * 512],
                            in0=psk[:], scalar1=WSCI, scalar2=None,
                            op0=ALU.mult)

                # ===== v projection -> vrow [k, dh] directly =====
                wvts = []
                for dc in range(NDC):
                    w = wop.tile([P, D], FP8, tag="wo8", name="wvt")
                    nc.sync.dma_start(w[:], wvt_d.ap()[li, dc])
                    wvts.append(w)
                for kc in range(NDC):
                    pv = [psp.tile([P, 512], F32, tag="mm", name="mm", bufs=4)
                          for _ in range(2)]
                    for dc in range(NDC):
                        for half in range(2):
                            mm(pv[half][:],
                               xT[dc][:, kc * P:(kc + 1) * P],
                               wvts[dc][:, half * 512:(half + 1) * 512],
                               first=(dc == 0), last=(dc == NDC - 1))
                    vv = vrow[kc].rearrange("p (h c) -> p h c", h=H)
                    for half in range(2):
                        hbase = half * 8
                        sv = pv[half].rearrange("p (h c) -> p h c", h=8)
                        nc.vector.tensor_scalar(
                            out=vv[:, hbase:hbase + 8, 0:64], in0=sv[:],
                            scalar1=WSCI, scalar2=None, op0=ALU.mult)

                # ===== attention (transposed scores [k, q]) =====
                def emit_scores(h):
                    ocn, rsub = h // 2, 64 * (h % 2)
                    bdts = []
                    for qc in range(NQC):
                        w = (qc + 1) * P
                        bdt = bdtp.tile([P, 512], F32, tag="bdt", name="bdt")
                        base = h * NQ * SKR + qc * P * (SKR - 1) + 1023
                        nc.sync.dma_start(
                            bdt[:, 0:w],
                            _ap(skap, base, [[SKR - 1, P], [1, w]]))
                        bdts.append(bdt)
                    psts = []
                    for jc in range(8):
                        pst = psp.tile([P, 512], F32, tag="mm", name="mm",
                                       bufs=4)
                        mm(pst[:],
                           kT[ocn][rsub:rsub + 64, jc * P:(jc + 1) * P],
                           rq[ocn][rsub:rsub + 64, :],
                           first=True, last=(jc >= NQC))
                        psts.append(pst)
                    ats = []
                    for qc in range(NQC):
                        for jc in range(qc + 1):
                            nc.tensor.matmul(
                                psts[jc][:, qc * P:(qc + 1) * P],
                                bdts[qc][:, jc * P:(jc + 1) * P],
                                identf[:], is_transpose=True,
                                start=False, stop=(qc == NQC - 1))
                    for jc in range(8):
                        at = atTp.tile([P, 512], BF16, tag="atT", name="atT")
                        nc.scalar.activation(at[:], psts[jc][:], AF.Exp,
                                             bias=zb_t[:], scale=SCALE)
                        ats.append(at)
                    return ats

                def emit_av(h, ats):
                    ocn, rsub = h // 2, 64 * (h % 2)
                    # pav rows [0:65]: AV rows 0..63, Z (ones-col sum) row 64
                    pav = psp.tile([P, 512], F32, tag="pav", name="pav",
                                   bufs=2)
                    for jc in range(NDC):
                        mm(pav[0:65, :],
                           vrow[jc][:, h * 65:h * 65 + 65],
                           ats[jc][:],
                           first=(jc == 0), last=(jc == NDC - 1))
                    zrow = zrowp.tile([P, 512], F32, tag="zrow", name="zrow")
                    nc.vector.reciprocal(zrow[64:65, :], pav[64:65, :])
                    nc.sync.dma_start(_ap(zd.ap(), h * NQ, [[1, NQ]]),
                                      zrow[64:65, :])
                    zrt = zrtp.tile([64, 512], F32, tag="zrt", name="zrt")
                    nc.sync.dma_start(
                        zrt[:], _ap(zd.ap(), h * NQ, [[0, 64], [1, NQ]]))
                    tmpo = zrowp.tile([64, 512], BF16, tag="tmpo",
                                      name="tmpo")
                    nc.vector.tensor_tensor(tmpo[:], pav[0:64, :], zrt[:],
                                            op=ALU.mult)
                    nc.sync.dma_start(aoT[ocn][rsub:rsub + 64, :], tmpo[:])

                prev = None
                for h in range(H):
                    ats = emit_scores(h)
                    if prev is not None:
                        emit_av(h - 1, prev)
                    prev = ats
                emit_av(H - 1, prev)

                # ===== Wo + residual + LN1 + xT1 =====
                wo_t = []
                for dc in range(NDC):
                    w = wop.tile([P, D], FP8, tag="wo8", name="wo8")
                    nc.sync.dma_start(
                        w[:], wot.ap()[li, dc * P:(dc + 1) * P, :])
                    wo_t.append(w)
                def wo_finish(qc, pp):
                    for o2 in range(2):
                        sl = slice(o2 * 512, (o2 + 1) * 512)
                        nc.vector.tensor_add(x_row[qc][:, sl],
                                             x_row[qc][:, sl], pp[o2][:])
                    layernorm(x_row[qc])
                    xb = xwp.tile([P, D], BF16, tag="xb", name="xb")
                    nc.vector.tensor_copy(xb[:], x_row[qc][:])
                    transpose_to(xT1, xb, qc)

                pend = None
                for qc in range(NQC):
                    pp = [psp.tile([P, 512], F32, tag="mm", name="mm", bufs=4)
                          for _ in range(2)]
                    for dc in range(NDC):
                        for o2 in range(2):
                            mm(pp[o2][:], aoT[dc][:, qc * P:(qc + 1) * P],
                               wo_t[dc][:, o2 * 512:(o2 + 1) * 512],
                               first=(dc == 0), last=(dc == NDC - 1))
                    if pend is not None:
                        wo_finish(*pend)
                    pend = (qc, pp)
                wo_finish(*pend)

                # b2 pre-add into residual stream
                for qc in range(NQC):
                    nc.vector.tensor_add(x_row[qc][:], x_row[qc][:], b2_t[:])

                # ===== FFN =====
                for fh in range(2):
                    gts = []
                    for fc16 in range(16):
                        fc = fh * 16 + fc16
                        w1t_ = wTp.tile([P, D], BF16, tag="w1", name="w1")
                        nc.sync.dma_start(w1t_[:], w1r.ap()[li, fc])
                        ph = psp.tile([P, NQ], F32, tag="mm", name="mm",
                                      bufs=4)
                        for dc in range(NDC):
                            mm(ph[:], w1t_[:, dc * P:(dc + 1) * P],
                               xT1[dc][:],
                               first=(dc == 0), last=(dc == NDC - 1))
                        gt = gtp.tile([P, NQ], BF16, tag="gt", name="gt")
                        nc.scalar.activation(gt[:], ph[:], AF.Gelu,
                                             bias=b1_t[:, fc:fc + 1],
                                             scale=1.0)
                        gts.append(gt)
                    for fcg in range(2):
                        last_group = (fh == 1 and fcg == 1)
                        w2_t = []
                        for f8 in range(8):
                            w = wrhsp.tile([P, D], BF16, tag="wrhs",
                                           name="wrhs")
                            fc = fh * 16 + fcg * 8 + f8
                            nc.sync.dma_start(
                                w[:], w2t.ap()[li, fc * P:(fc + 1) * P, :])
                            w2_t.append(w)
                        for qc in range(NQC):
                            for o2 in range(2):
                                sl = slice(o2 * 512, (o2 + 1) * 512)
                                pf = psp.tile([P, 512], F32, tag="mm",
                                              name="mm", bufs=4)
                                for f8 in range(8):
                                    mm(pf[:],
                                       gts[fcg * 8 + f8][:,
                                                         qc * P:(qc + 1) * P],
                                       w2_t[f8][:, sl],
                                       first=(f8 == 0), last=(f8 == 7))
                                nc.vector.tensor_add(x_row[qc][:, sl],
                                                     x_row[qc][:, sl],
                                                     pf[:])
                            if last_group:
                                # LN2 for this qc while later qcs' matmuls run
                                layernorm(x_row[qc])
                                if li == NL - 1:
                                    nc.sync.dma_start(
                                        yout.ap()[qc * P:(qc + 1) * P, :],
                                        x_row[qc][:])
                                else:
                                    xb = xwp.tile([P, D], BF16, tag="xb",
                                                  name="xb")
                                    nc.vector.tensor_copy(xb[:], x_row[qc][:])
                                    transpose_to(xTown, xb, qc)
                if li < NL - 1:
                    for dc in range(NDC):
                        nc.sync.dma_start(
                            _ap(agin.ap(), dc * P * NQ, [[NQ, P], [1, NQ]]),
                            xTown[dc][:])

    nc.finalize()
    return nc


def _prep_host(inputs):
    bf = ml_dtypes.bfloat16
    embed = np.asarray(inputs["embed"], np.float32)
    seq = np.asarray(inputs["seq"]).astype(np.int64)
    x0 = embed[seq]                                   # [B, S, D] f32

    Wq = np.asarray(inputs["Wq"], np.float32)
    Wk = np.asarray(inputs["Wk"], np.float32)
    Wv = np.asarray(inputs["Wv"], np.float32)
    Wo = np.asarray(inputs["Wo"], np.float32)
    w1 = np.asarray(inputs["w1"], np.float32)
    w2 = np.asarray(inputs["w2"], np.float32)
    b1 = np.asarray(inputs["b1"], np.float32)
    b2 = np.asarray(inputs["b2"], np.float32)
    r_emb = np.asarray(inputs["r_emb"], np.float32)
    r_w_bias = np.asarray(inputs["r_w_bias"], np.float32)
    r_bias = np.asarray(inputs["r_bias"], np.float32)

    f8 = ml_dtypes.float8_e3m4

    def packl(WT):   # [D, D] -> [NDC, P, D] lhsT pack
        return np.ascontiguousarray(
            WT.reshape(NDC, P, NDC, P).transpose(2, 1, 0, 3)
            .reshape(NDC, P, D))

    wproj = (np.stack([
        np.stack([packl(Wq[l].T), packl(Wk[l].T)])
        for l in range(NL)]) * WSC).astype(f8)
    wvt = (np.stack([Wv[l].T.reshape(NDC, P, D)
                     for l in range(NL)]) * WSC).astype(f8)
    wot = (np.stack([Wo[l].T for l in range(NL)]) * WSC).astype(f8)
    w1r = np.stack([
        np.ascontiguousarray(
            w1[l].T.reshape(NDC, P, NFC, P).transpose(2, 1, 0, 3)
            .reshape(NFC, P, D))
        for l in range(NL)]).astype(bf)
    w2t = np.stack([w2[l].T for l in range(NL)]).astype(bf)
    b1r = np.stack([b1[l].reshape(NFC, P).T for l in range(NL)])
    b1r = np.ascontiguousarray(b1r).astype(np.float32)
    b2r = b2.astype(np.float32)
    rwbr = np.stack([r_w_bias[l].reshape(D).reshape(NDC, P).T
                     for l in range(NL)])
    rwbr = np.ascontiguousarray(rwbr).astype(np.float32)

    # rep: per head-pair stacked re^T; rb_adj = rb - rwb @ re^T separately
    off = r_emb.shape[2] - S     # MAX_KLEN - S
    rep = np.empty((NL, NDC, P, S), np.float32)
    rba = np.empty((NL, H, S), np.float32)
    for l in range(NL):
        for h in range(H):
            re = r_emb[l, h, off:, :]            # [S, DH]
            rep[l, h // 2, (h % 2) * 64:(h % 2) * 64 + 64] = re.T
            rba[l, h] = r_bias[l, h, off:] - r_w_bias[l, h] @ re.T

    ident = np.eye(P, dtype=bf)
    identf = np.eye(P, dtype=np.float32)

    in_maps = []
    for c in range(NCORES):
        b, half = c // 2, c % 2
        q0 = half * NQ
        xr = np.ascontiguousarray(x0[b, q0:q0 + NQ]).astype(np.float32)
        xt = np.ascontiguousarray(x0[b, q0:q0 + NQ].T).astype(bf)
        repc = np.roll(rep, q0, axis=-1) if q0 else rep
        rbac = np.roll(rba, q0, axis=-1) if q0 else rba
        repc = np.ascontiguousarray(repc[..., 512:]).astype(bf)
        rbac = np.ascontiguousarray(rbac[..., 512:]).astype(bf)
        in_maps.append({
            "xrow0": xr, "xt0": xt, "wproj": wproj, "wvt": wvt, "wot": wot,
            "w1r": w1r, "w2t": w2t, "b1r": b1r, "b2r": b2r,
            "repd": repc, "rbd": rbac, "rwbr": rwbr, "ident": ident,
            "identf": identf,
        })
    return in_maps


def run(inputs, trace=False):
    if "nc" not in _cache:
        _cache["nc"] = build()
    nc = _cache["nc"]
    in_maps = _prep_host(inputs)
    res = run_bass_kernel_spmd(nc, in_maps, list(range(NCORES)),
                               trace=trace)
    y = np.zeros((B, S, D), np.float32)
    for c in range(NCORES):
        b, half = c // 2, c % 2
        y[b, half * NQ:(half + 1) * NQ] = res.results[c]["y"]
    return y, res


def kernel(**inputs) -> np.ndarray:
    y, _ = run(inputs)
    return y


# revision 14
# speedup vs baseline: 2.2245x; 1.0051x over previous
"""Trainium2 Bass kernel for nn_CompressiveEncoder (4-layer relative-position
transformer encoder, B=4 S=1024 D=1024 H=16 FF=4096).

Sharding: 8 cores = (batch b = c//2) x (query-half q0 = (c%2)*512).
Each layer AllGathers the bf16 transposed hidden state; each core selects its
batch pair via indirect DMA, computes K/V for its full batch and attention +
FFN for its own 512 query rows.

Attention is computed in transposed [k, q] orientation: AC^T comes straight
from kT/rq matmuls, and the Music-Transformer shift term BD is accumulated
into the same PSUM banks with transpose-mode matmuls reading skewed DRAM
tiles (f32).  The skew is restricted to its lower-triangular support
(core-local row index >= key index), matching the baseline semantics.
Softmax normalization is folded into the attention@V matmul via a ones
column appended to V, so exp() outputs are consumed unnormalized and each
head is scaled by 1/Z once on the [64, 512] output.
"""

import sys

sys.path.insert(0, "/opt/trn_rl_repo")

import numpy as np
import ml_dtypes

import concourse.bass as bass
import concourse.mybir as mybir
import concourse.tile as tile
from concourse import bacc
from concourse.bass_utils import run_bass_kernel_spmd

BF16 = mybir.dt.bfloat16
F32 = mybir.dt.float32
FP8 = mybir.dt.float8e3
AF = mybir.ActivationFunctionType
ALU = mybir.AluOpType

NL, D, H, DH, S, FF_DIM = 4, 1024, 16, 64, 1024, 4096
B = 4
NCORES = 8
NQ = 512           # query rows per core
P = 128
SCALE = float(H) ** -0.5   # reference scales by 1/sqrt(heads) = 0.25
EPS = 1e-5
NQC = NQ // P      # 4
NDC = D // P       # 8
NFC = FF_DIM // P  # 32
SKR = 1152         # skew row width (f32 elements): 1024 data + 128 zero pad
WSC = 64.0         # fp8 e3m4 weight pre-scale for q/k/v/o projections
WSCI = 1.0 / WSC

_cache: dict = {}


def _ap(t, off, pattern):
    return bass.AP(tensor=t.tensor, offset=t.offset + off, ap=pattern)


def build():
    nc = bacc.Bacc("TRN2", target_bir_lowering=False, debug=False,
                   num_devices=NCORES)

    xrow0 = nc.dram_tensor("xrow0", [NQ, D], F32, kind="ExternalInput")
    xt0 = nc.dram_tensor("xt0", [D, NQ], BF16, kind="ExternalInput")
    # q/k lhsT packs
    wproj = nc.dram_tensor("wproj", [NL, 2, NDC, P, D], FP8,
                           kind="ExternalInput")
    wvt_d = nc.dram_tensor("wvt", [NL, NDC, P, D], FP8,
                           kind="ExternalInput")     # Wv.T rows (rhs pack)
    wot = nc.dram_tensor("wot", [NL, D, D], FP8, kind="ExternalInput")
    w1r = nc.dram_tensor("w1r", [NL, NFC, P, D], BF16, kind="ExternalInput")
    w2t = nc.dram_tensor("w2t", [NL, FF_DIM, D], BF16, kind="ExternalInput")
    b1r = nc.dram_tensor("b1r", [NL, P, NFC], F32, kind="ExternalInput")
    b2r = nc.dram_tensor("b2r", [NL, D], F32, kind="ExternalInput")
    repd = nc.dram_tensor("repd", [NL, NDC, P, 512], BF16,
                          kind="ExternalInput")      # rolled re^T, last 512 js
    rbd = nc.dram_tensor("rbd", [NL, H, 512], BF16,
                         kind="ExternalInput")       # rolled rb adj, last 512
    rwbr = nc.dram_tensor("rwbr", [NL, P, NDC], F32, kind="ExternalInput")
    ident_d = nc.dram_tensor("ident", [P, P], BF16, kind="ExternalInput")
    identf_d = nc.dram_tensor("identf", [P, P], F32, kind="ExternalInput")
    yout = nc.dram_tensor("y", [NQ, D], F32, kind="ExternalOutput")

    # internal DRAM
    sk = nc.dram_tensor("sk_buf", [H * NQ * SKR], F32)
    zd = nc.dram_tensor("zd_buf", [H * NQ], F32)
    agin = nc.dram_tensor("agin_buf", [D * NQ], BF16)
    agout = nc.dram_tensor("agout_buf", [2 * D, NQ], BF16)

    from contextlib import ExitStack
    with tile.TileContext(nc) as tc, ExitStack() as stk:
            singles = stk.enter_context(tc.tile_pool(name="singles", bufs=1))
            wTp = stk.enter_context(tc.tile_pool(name="wT", bufs=3))
            wqp = stk.enter_context(tc.tile_pool(name="wqp", bufs=8))
            # (funded by shrinking gt pool)
            wrhsp = stk.enter_context(tc.tile_pool(name="wrhs", bufs=9))
            wop = stk.enter_context(tc.tile_pool(name="wo", bufs=9))
            gtp = stk.enter_context(tc.tile_pool(name="gt", bufs=16))
            atTp = stk.enter_context(tc.tile_pool(name="atT", bufs=14))
            bdtp = stk.enter_context(tc.tile_pool(name="bdt", bufs=5))
            brawp = stk.enter_context(tc.tile_pool(name="braw", bufs=2))
            repp = stk.enter_context(tc.tile_pool(name="rep", bufs=2))
            rbp = stk.enter_context(tc.tile_pool(name="rb", bufs=2))
            zrowp = stk.enter_context(tc.tile_pool(name="zrow", bufs=2))
            zrtp = stk.enter_context(tc.tile_pool(name="zrt", bufs=2))
            xwp = stk.enter_context(tc.tile_pool(name="xw", bufs=2))
            smallp = stk.enter_context(tc.tile_pool(name="small", bufs=16))
            psp = stk.enter_context(
                tc.tile_pool(name="psum", bufs=1, space="PSUM"))
            # ------------- persistent SBUF state -------------
            x_row = [singles.tile([P, D], F32, tag=f"xrow{i}", name=f"xrow{i}")
                     for i in range(NQC)]
            xTown = [singles.tile([P, NQ], BF16, tag=f"xto{i}", name=f"xto{i}")
                     for i in range(NDC)]       # my own x^T (this layer's in)
            xT = [singles.tile([P, 2 * NQ], BF16, tag=f"xt{i}", name=f"xt{i}")
                  for i in range(NDC)]          # gathered x^T, my full batch
            xT1 = [singles.tile([P, NQ], BF16, tag=f"xt1_{i}", name=f"xt1_{i}")
                   for i in range(NDC)]         # post-LN1 x^T, my rows
            kT = [singles.tile([P, S], BF16, tag=f"kt{i}", name=f"kt{i}")
                  for i in range(NDC)]
            # v rows + per-head ones column (even head: [dh,1], odd: [1,dh])
            vrow = [singles.tile([P, H * 65], BF16, tag=f"vr{i}",
                                 name=f"vr{i}")
                    for i in range(NDC)]
            rq = [singles.tile([P, NQ], BF16, tag=f"rq{i}", name=f"rq{i}")
                  for i in range(NDC)]
            aoT = [singles.tile([P, NQ], BF16, tag=f"aoT{i}", name=f"aoT{i}")
                   for i in range(NDC)]
            ident = singles.tile([P, P], BF16, tag="ident", name="ident")
            identf = singles.tile([P, P], F32, tag="identf", name="identf")
            eps_t = singles.tile([P, 1], F32, tag="eps", name="eps")
            zb_t = singles.tile([P, 1], F32, tag="zbias", name="zbias")
            b2_t = singles.tile([P, D], F32, tag="b2rep", name="b2rep")
            b1_t = singles.tile([P, NFC], F32, tag="b1", name="b1")
            rwb_t = singles.tile([P, NDC], F32, tag="rwb", name="rwb")
            zeros_t = singles.tile([P, P], F32, tag="zeros", name="zeros")

            nc.sync.dma_start(ident[:], ident_d.ap())
            nc.sync.dma_start(identf[:], identf_d.ap())
            nc.vector.memset(eps_t[:], EPS)
            nc.vector.memset(zb_t[:], 0.0)
            nc.vector.memset(zeros_t[:], 0.0)
            # ones column in vrow: head h -> col h*65+64 (for softmax Z)
            for dc in range(NDC):
                vv = vrow[dc].rearrange("p (h c) -> p h c", h=H)
                nc.vector.memset(vv[:, :, 64:65], WSC)
            skap = sk.ap()
            # zero skew pad cols [1024, 1152) once
            for blk in range(H * NQ // P):
                dst = _ap(skap, blk * P * SKR + 1024, [[SKR, P], [1, P]])
                nc.sync.dma_start(dst, zeros_t[:, :])
            for qc in range(NQC):
                nc.sync.dma_start(x_row[qc][:],
                                  xrow0.ap()[qc * P:(qc + 1) * P, :])
            for dc in range(NDC):
                nc.sync.dma_start(xTown[dc][:],
                                  xt0.ap()[dc * P:(dc + 1) * P, :])
            nc.sync.dma_start(agin.ap(), xt0.ap())

            def mm(out, lhsT, rhs, first=True, last=True):
                nc.tensor.matmul(out, lhsT, rhs, start=first, stop=last)

            def layernorm(xr):
                st = smallp.tile([P, 2, 6], F32, tag="bnst", name="bnst")
                nc.vector.bn_stats(st[:, 0, :], xr[:, 0:512])
                nc.vector.bn_stats(st[:, 1, :], xr[:, 512:1024])
                mv = smallp.tile([P, 2], F32, tag="bnmv", name="bnmv")
                nc.vector.bn_aggr(mv[:], st[:])
                sd = smallp.tile([P, 1], F32, tag="sd", name="sd")
                nc.scalar.activation(sd[:], mv[:, 1:2], AF.Sqrt,
                                     bias=eps_t[:], scale=1.0)
                rs = smallp.tile([P, 1], F32, tag="rs", name="rs")
                nc.vector.reciprocal(rs[:], sd[:])
                nc.vector.tensor_scalar(
                    out=xr[:], in0=xr[:], scalar1=mv[:, 0:1],
                    scalar2=rs[:], op0=ALU.subtract, op1=ALU.mult)

            def transpose_to(dsts, src_bf16, qc):
                """src [128(q), 1024(d)] -> dsts[dc][:, qc*128:+128]."""
                for g in range(2):
                    pt = psp.tile([P, 512], BF16, tag="pt", name="pt",
                                  bufs=2, padded_shape=[P, 1024])
                    for k4 in range(4):
                        dc = g * 4 + k4
                        nc.tensor.transpose(
                            pt[:, k4 * P:(k4 + 1) * P],
                            src_bf16[:, dc * P:(dc + 1) * P], ident[:])
                    for k4 in range(4):
                        dc = g * 4 + k4
                        nc.vector.tensor_copy(
                            dsts[dc][:, qc * P:(qc + 1) * P],
                            pt[:, k4 * P:(k4 + 1) * P])

            for li in range(NL):
                # ===== allgather x^T (async; overlapped by q-proj + shift) ==
                nc.gpsimd.collective_compute(
                    "AllGather", ALU.bypass,
                    replica_groups=[[2 * g, 2 * g + 1]
                                    for g in range(NCORES // 2)],
                    ins=[agin.ap()], outs=[agout.ap()],
                )

                nc.sync.dma_start(rwb_t[:], rwbr.ap()[li])
                nc.sync.dma_start(b1_t[:], b1r.ap()[li])
                nc.sync.dma_start(
                    b2_t[:], _ap(b2r.ap(), li * D, [[0, P], [1, D]]))

                # ===== q projection (own rows; AG-independent) =====
                for oc in range(NDC):
                    wq = wqp.tile([P, D], FP8, tag="wq", name="wq")
                    nc.sync.dma_start(wq[:], wproj.ap()[li, 0, oc])
                    ps = psp.tile([P, NQ], F32, tag="mm", name="mm", bufs=4)
                    for dc in range(NDC):
                        mm(ps[:], wq[:, dc * P:(dc + 1) * P], xTown[dc][:],
                           first=(dc == 0), last=(dc == NDC - 1))
                    nc.vector.tensor_scalar(
                        out=rq[oc][:], in0=ps[:],
                        scalar1=WSCI, scalar2=rwb_t[:, oc:oc + 1],
                        op0=ALU.mult, op1=ALU.add)

                # ===== B~ scores + skew write (emitted per-head below) ====
                rept_c = {}

                def emit_b(h):
                    ocn, rsub = h // 2, 64 * (h % 2)
                    if h % 2 == 0:
                        rept = repp.tile([P, 512], BF16, tag="rep",
                                         name="rep")
                        nc.sync.dma_start(rept[:], repd.ap()[li, ocn])
                        rept_c[ocn] = rept
                    rept = rept_c[ocn]
                    rbt = rbp.tile([P, 512], BF16, tag="rb", name="rb")
                    nc.sync.dma_start(
                        rbt[:], _ap(rbd.ap(), (li * H + h) * 512,
                                    [[0, P], [1, 512]]))
                    for qc in range(NQC):
                        w = (qc + 1) * P
                        psb = psp.tile([P, NQ], F32, tag="mm", name="mm",
                                       bufs=4)
                        mm(psb[:, 0:w],
                           rq[ocn][rsub:rsub + 64, qc * P:(qc + 1) * P],
                           rept[rsub:rsub + 64, 512 - w:512])
                        braw = brawp.tile([P, 512], F32, tag="braw",
                                          name="braw")
                        nc.vector.tensor_tensor(
                            braw[:, 0:w], psb[:, 0:w],
                            rbt[:, 512 - w:512], op=ALU.add)
                        base = h * NQ * SKR + qc * P * SKR + (896 - qc * P)
                        nc.sync.dma_start(
                            _ap(skap, base, [[SKR, P], [1, w]]),
                            braw[:, 0:w])

                # ===== gather my batch pair (waits on AG) =====
                for dc in range(NDC):
                    for half in range(2):
                        nc.sync.dma_start(
                            xT[dc][:, half * NQ:(half + 1) * NQ],
                            agout.ap()[half * D + dc * P:
                                       half * D + (dc + 1) * P, :])

                # ===== k projection -> kT [dh, k] =====
                for oc in range(NDC):
                    wk = wqp.tile([P, D], FP8, tag="wq", name="wq")
                    nc.sync.dma_start(wk[:], wproj.ap()[li, 1, oc])
                    for jh in range(2):
                        psk = psp.tile([P, 512], F32, tag="mm", name="mm",
                                       bufs=4)
                        for dc in range(NDC):
                            mm(psk[:],
                               wk[:, dc * P:(dc + 1) * P],
                               xT[dc][:, jh * 512:(jh + 1) * 512],
                               first=(dc == 0), last=(dc == NDC - 1))
                        nc.vector.tensor_scalar(
                            out=kT[oc][:, jh * 512:(jh + 1) * 512],
                            in0=psk[:], scalar1=WSCI, scalar2=None,
                            op0=ALU.mult)

                # ===== v projection -> vrow [k, dh] directly =====
                wvts = []
                for dc in range(NDC):
                    w = wop.tile([P, D], FP8, tag="wo8", name="wvt")
                    nc.sync.dma_start(w[:], wvt_d.ap()[li, dc])
                    wvts.append(w)
                for kc in range(NDC):
                    pv = [psp.tile([P, 512], F32, tag="mm", name="mm", bufs=4)
                          for _ in range(2)]
                    for dc in range(NDC):
                        for half in range(2):
                            mm(pv[half][:],
                               xT[dc][:, kc * P:(kc + 1) * P],
                               wvts[dc][:, half * 512:(half + 1) * 512],
                               first=(dc == 0), last=(dc == NDC - 1))
                    vv = vrow[kc].rearrange("p (h c) -> p h c", h=H)
                    for half in range(2):
                        hbase = half * 8
                        sv = pv[half].rearrange("p (h c) -> p h c", h=8)
                        nc.vector.tensor_scalar(
                            out=vv[:, hbase:hbase + 8, 0:64], in0=sv[:],
                            scalar1=WSCI, scalar2=None, op0=ALU.mult)

                # ===== attention (transposed scores [k, q]) =====
                def emit_scores(h):
                    ocn, rsub = h // 2, 64 * (h % 2)
                    bdts = []
                    for qc in range(NQC):
                        w = (qc + 1) * P
                        bdt = bdtp.tile([P, 512], F32, tag="bdt", name="bdt")
                        base = h * NQ * SKR + qc * P * (SKR - 1) + 1023
                        nc.sync.dma_start(
                            bdt[:, 0:w],
                            _ap(skap, base, [[SKR - 1, P], [1, w]]))
                        bdts.append(bdt)
                    psts = []
                    for jc in range(8):
                        pst = psp.tile([P, 512], F32, tag="mm", name="mm",
                                       bufs=4)
                        mm(pst[:],
                           kT[ocn][rsub:rsub + 64, jc * P:(jc + 1) * P],
                           rq[ocn][rsub:rsub + 64, :],
                           first=True, last=(jc >= NQC))
                        psts.append(pst)
                    ats = []
                    for qc in range(NQC):
                        for jc in range(qc + 1):
                            nc.tensor.matmul(
                                psts[jc][:, qc * P:(qc + 1) * P],
                                bdts[qc][:, jc * P:(jc + 1) * P],
                                identf[:], is_transpose=True,
                                start=False, stop=(qc == NQC - 1))
                    for jc in range(8):
                        at = atTp.tile([P, 512], BF16, tag="atT", name="atT")
                        nc.scalar.activation(at[:], psts[jc][:], AF.Exp,
                                             bias=zb_t[:], scale=SCALE)
                        ats.append(at)
                    return ats

                def emit_av(h, ats):
                    ocn, rsub = h // 2, 64 * (h % 2)
                    # pav rows [0:65]: AV rows 0..63, Z (ones-col sum) row 64
                    pav = psp.tile([P, 512], F32, tag="pav", name="pav",
                                   bufs=2)
                    for jc in range(NDC):
                        mm(pav[0:65, :],
                           vrow[jc][:, h * 65:h * 65 + 65],
                           ats[jc][:],
                           first=(jc == 0), last=(jc == NDC - 1))
                    zrow = zrowp.tile([P, 512], F32, tag="zrow", name="zrow")
                    nc.vector.reciprocal(zrow[64:65, :], pav[64:65, :])
                    nc.sync.dma_start(_ap(zd.ap(), h * NQ, [[1, NQ]]),
                                      zrow[64:65, :])
                    zrt = zrtp.tile([64, 512], F32, tag="zrt", name="zrt")
                    nc.sync.dma_start(
                        zrt[:], _ap(zd.ap(), h * NQ, [[0, 64], [1, NQ]]))
                    tmpo = zrowp.tile([64, 512], BF16, tag="tmpo",
                                      name="tmpo")
                    nc.vector.tensor_tensor(tmpo[:], pav[0:64, :], zrt[:],
                                            op=ALU.mult)
                    nc.sync.dma_start(aoT[ocn][rsub:rsub + 64, :], tmpo[:])

                emit_b(0)
                emit_b(1)
                prev = None
                for h in range(H):
                    if h + 2 < H:
                        emit_b(h + 2)
                    ats = emit_scores(h)
                    if prev is not None:
                        emit_av(h - 1, prev)
                    prev = ats
                emit_av(H - 1, prev)

                # ===== Wo + residual + LN1 + xT1 =====
                wo_t = []
                for dc in range(NDC):
                    w = wop.tile([P, D], FP8, tag="wo8", name="wo8")
                    nc.sync.dma_start(
                        w[:], wot.ap()[li, dc * P:(dc + 1) * P, :])
                    wo_t.append(w)
                def wo_finish(qc, pp):
                    for o2 in range(2):
                        sl = slice(o2 * 512, (o2 + 1) * 512)
                        nc.vector.tensor_add(x_row[qc][:, sl],
                                             x_row[qc][:, sl], pp[o2][:])
                    layernorm(x_row[qc])
                    xb = xwp.tile([P, D], BF16, tag="xb", name="xb")
                    nc.vector.tensor_copy(xb[:], x_row[qc][:])
                    transpose_to(xT1, xb, qc)

                pend = None
                for qc in range(NQC):
                    pp = [psp.tile([P, 512], F32, tag="mm", name="mm", bufs=4)
                          for _ in range(2)]
                    for dc in range(NDC):
                        for o2 in range(2):
                            mm(pp[o2][:], aoT[dc][:, qc * P:(qc + 1) * P],
                               wo_t[dc][:, o2 * 512:(o2 + 1) * 512],
                               first=(dc == 0), last=(dc == NDC - 1))
                    if pend is not None:
                        wo_finish(*pend)
                    pend = (qc, pp)
                wo_finish(*pend)

                # b2 pre-add into residual stream
                for qc in range(NQC):
                    nc.vector.tensor_add(x_row[qc][:], x_row[qc][:], b2_t[:])

                # ===== FFN =====
                for fh in range(2):
                    gts = []
                    for fc16 in range(16):
                        fc = fh * 16 + fc16
                        w1t_ = wTp.tile([P, D], BF16, tag="w1", name="w1")
                        nc.sync.dma_start(w1t_[:], w1r.ap()[li, fc])
                        ph = psp.tile([P, NQ], F32, tag="mm", name="mm",
                                      bufs=4)
                        for dc in range(NDC):
                            mm(ph[:], w1t_[:, dc * P:(dc + 1) * P],
                               xT1[dc][:],
                               first=(dc == 0), last=(dc == NDC - 1))
                        gt = gtp.tile([P, NQ], BF16, tag="gt", name="gt")
                        nc.scalar.activation(gt[:], ph[:], AF.Gelu,
                                             bias=b1_t[:, fc:fc + 1],
                                             scale=1.0)
                        gts.append(gt)
                    for fcg in range(2):
                        last_group = (fh == 1 and fcg == 1)
                        w2_t = []
                        for f8 in range(8):
                            w = wrhsp.tile([P, D], BF16, tag="wrhs",
                                           name="wrhs")
                            fc = fh * 16 + fcg * 8 + f8
                            nc.sync.dma_start(
                                w[:], w2t.ap()[li, fc * P:(fc + 1) * P, :])
                            w2_t.append(w)
                        for qc in range(NQC):
                            for o2 in range(2):
                                sl = slice(o2 * 512, (o2 + 1) * 512)
                                pf = psp.tile([P, 512], F32, tag="mm",
                                              name="mm", bufs=4)
                                for f8 in range(8):
                                    mm(pf[:],
                                       gts[fcg * 8 + f8][:,
                                                         qc * P:(qc + 1) * P],
                                       w2_t[f8][:, sl],
                                       first=(f8 == 0), last=(f8 == 7))
                                nc.vector.tensor_add(x_row[qc][:, sl],
                                                     x_row[qc][:, sl],
                                                     pf[:])
                            if last_group:
                                # LN2 for this qc while later qcs' matmuls run
                                layernorm(x_row[qc])
                                if li == NL - 1:
                                    nc.sync.dma_start(
                                        yout.ap()[qc * P:(qc + 1) * P, :],
                                        x_row[qc][:])
                                else:
                                    xb = xwp.tile([P, D], BF16, tag="xb",
                                                  name="xb")
                                    nc.vector.tensor_copy(xb[:], x_row[qc][:])
                                    transpose_to(xTown, xb, qc)
                if li < NL - 1:
                    for dc in range(NDC):
                        nc.sync.dma_start(
                            _ap(agin.ap(), dc * P * NQ, [[NQ, P], [1, NQ]]),
                            xTown[dc][:])

    nc.finalize()
    return nc


def _prep_host(inputs):
    bf = ml_dtypes.bfloat16
    embed = np.asarray(inputs["embed"], np.float32)
    seq = np.asarray(inputs["seq"]).astype(np.int64)
    x0 = embed[seq]                                   # [B, S, D] f32

    Wq = np.asarray(inputs["Wq"], np.float32)
    Wk = np.asarray(inputs["Wk"], np.float32)
    Wv = np.asarray(inputs["Wv"], np.float32)
    Wo = np.asarray(inputs["Wo"], np.float32)
    w1 = np.asarray(inputs["w1"], np.float32)
    w2 = np.asarray(inputs["w2"], np.float32)
    b1 = np.asarray(inputs["b1"], np.float32)
    b2 = np.asarray(inputs["b2"], np.float32)
    r_emb = np.asarray(inputs["r_emb"], np.float32)
    r_w_bias = np.asarray(inputs["r_w_bias"], np.float32)
    r_bias = np.asarray(inputs["r_bias"], np.float32)

    f8 = ml_dtypes.float8_e3m4

    def packl(WT):   # [D, D] -> [NDC, P, D] lhsT pack
        return np.ascontiguousarray(
            WT.reshape(NDC, P, NDC, P).transpose(2, 1, 0, 3)
            .reshape(NDC, P, D))

    wproj = (np.stack([
        np.stack([packl(Wq[l].T), packl(Wk[l].T)])
        for l in range(NL)]) * WSC).astype(f8)
    wvt = (np.stack([Wv[l].T.reshape(NDC, P, D)
                     for l in range(NL)]) * WSC).astype(f8)
    wot = (np.stack([Wo[l].T for l in range(NL)]) * WSC).astype(f8)
    w1r = np.stack([
        np.ascontiguousarray(
            w1[l].T.reshape(NDC, P, NFC, P).transpose(2, 1, 0, 3)
            .reshape(NFC, P, D))
        for l in range(NL)]).astype(bf)
    w2t = np.stack([w2[l].T for l in range(NL)]).astype(bf)
    b1r = np.stack([b1[l].reshape(NFC, P).T for l in range(NL)])
    b1r = np.ascontiguousarray(b1r).astype(np.float32)
    b2r = b2.astype(np.float32)
    rwbr = np.stack([r_w_bias[l].reshape(D).reshape(NDC, P).T
                     for l in range(NL)])
    rwbr = np.ascontiguousarray(rwbr).astype(np.float32)

    # rep: per head-pair stacked re^T; rb_adj = rb - rwb @ re^T separately
    off = r_emb.shape[2] - S     # MAX_KLEN - S
    rep = np.empty((NL, NDC, P, S), np.float32)
    rba = np.empty((NL, H, S), np.float32)
    for l in range(NL):
        for h in range(H):
            re = r_emb[l, h, off:, :]            # [S, DH]
            rep[l, h // 2, (h % 2) * 64:(h % 2) * 64 + 64] = re.T
            rba[l, h] = r_bias[l, h, off:] - r_w_bias[l, h] @ re.T

    ident = np.eye(P, dtype=bf)
    identf = np.eye(P, dtype=np.float32)

    in_maps = []
    for c in range(NCORES):
        b, half = c // 2, c % 2
        q0 = half * NQ
        xr = np.ascontiguousarray(x0[b, q0:q0 + NQ]).astype(np.float32)
        xt = np.ascontiguousarray(x0[b, q0:q0 + NQ].T).astype(bf)
        repc = np.roll(rep, q0, axis=-1) if q0 else rep
        rbac = np.roll(rba, q0, axis=-1) if q0 else rba
        repc = np.ascontiguousarray(repc[..., 512:]).astype(bf)
        rbac = np.ascontiguousarray(rbac[..., 512:]).astype(bf)
        in_maps.append({
            "xrow0": xr, "xt0": xt, "wproj": wproj, "wvt": wvt, "wot": wot,
            "w1r": w1r, "w2t": w2t, "b1r": b1r, "b2r": b2r,
            "repd": repc, "rbd": rbac, "rwbr": rwbr, "ident": ident,
            "identf": identf,
        })
    return in_maps


def run(inputs, trace=False):
    if "nc" not in _cache:
        _cache["nc"] = build()
    nc = _cache["nc"]
    in_maps = _prep_host(inputs)
    res = run_bass_kernel_spmd(nc, in_maps, list(range(NCORES)),
                               trace=trace)
    y = np.zeros((B, S, D), np.float32)
    for c in range(NCORES):
        b, half = c // 2, c % 2
        y[b, half * NQ:(half + 1) * NQ] = res.results[c]["y"]
    return y, res


def kernel(**inputs) -> np.ndarray:
    y, _ = run(inputs)
    return y


# revision 17
# speedup vs baseline: 2.3419x; 1.0527x over previous
"""Trainium2 Bass kernel for nn_CompressiveEncoder (4-layer relative-position
transformer encoder, B=4 S=1024 D=1024 H=16 FF=4096).

Sharding: 8 cores = (batch b = c//2) x (query-half q0 = (c%2)*512).
Each layer pairwise-AllGathers the bf16 transposed hidden state within a
batch pair, computes K/V for the full batch row and attention + FFN for its
own 512 query rows.

Attention is computed in transposed [k, q] orientation: AC^T comes straight
from kT/rq matmuls, and the Music-Transformer shift term BD is accumulated
into the same PSUM banks with transpose-mode matmuls reading skewed DRAM
tiles (f32).  The skew is restricted to its lower-triangular support
(core-local row index >= key index), matching the baseline semantics.
Softmax normalization is folded into the attention@V matmul via a
WSC-scaled ones column appended to V; each head is scaled by 1/Z once on
the [64, 512] AV output, which also folds the fp8 weight descale for Wo.
Projection weights (Wq/Wk/Wv/Wo) are fp8-e3m4, pre-scaled by WSC=64 on the
host; descales fold into existing vector ops.
"""

import sys

sys.path.insert(0, "/opt/trn_rl_repo")

from contextlib import ExitStack

import numpy as np
import ml_dtypes

import concourse.bass as bass
import concourse.mybir as mybir
import concourse.tile as tile
from concourse import bacc
from concourse.bass_utils import run_bass_kernel_spmd

BF16 = mybir.dt.bfloat16
F32 = mybir.dt.float32
FP8 = mybir.dt.float8e3
AF = mybir.ActivationFunctionType
ALU = mybir.AluOpType

NL, D, H, DH, S, FF_DIM = 4, 1024, 16, 64, 1024, 4096
B = 4
NCORES = 8
NQ = 512           # query rows per core
P = 128
SCALE = float(H) ** -0.5   # reference scales by 1/sqrt(heads) = 0.25
EPS = 1e-5
NQC = NQ // P      # 4
NDC = D // P       # 8
NFC = FF_DIM // P  # 32
SKR = 1152         # skew row width (f32 elements): 1024 data + 128 zero pad
WSC = 64.0         # fp8 e3m4 weight pre-scale for q/k/v/o projections
WSCI = 1.0 / WSC

_cache: dict = {}


def _ap(t, off, pattern):
    return bass.AP(tensor=t.tensor, offset=t.offset + off, ap=pattern)


def build():
    nc = bacc.Bacc("TRN2", target_bir_lowering=False, debug=False,
                   num_devices=NCORES)

    xrow0 = nc.dram_tensor("xrow0", [NQ, D], F32, kind="ExternalInput")
    xt0 = nc.dram_tensor("xt0", [D, NQ], BF16, kind="ExternalInput")
    wproj = nc.dram_tensor("wproj", [NL, 2, NDC, P, D], FP8,
                           kind="ExternalInput")     # q/k lhsT packs
    wvt_d = nc.dram_tensor("wvt", [NL, NDC, P, D], FP8,
                           kind="ExternalInput")     # Wv.T rows (rhs pack)
    wot = nc.dram_tensor("wot", [NL, D, D], FP8, kind="ExternalInput")
    w1r = nc.dram_tensor("w1r", [NL, NFC, P, D], BF16, kind="ExternalInput")
    w2t = nc.dram_tensor("w2t", [NL, FF_DIM, D], BF16, kind="ExternalInput")
    b1r = nc.dram_tensor("b1r", [NL, P, NFC], F32, kind="ExternalInput")
    b2r = nc.dram_tensor("b2r", [NL, D], F32, kind="ExternalInput")
    repd = nc.dram_tensor("repd", [NL, NDC, P, 512], BF16,
                          kind="ExternalInput")      # rolled re^T, last 512
    rbd = nc.dram_tensor("rbd", [NL, H, 512], BF16,
                         kind="ExternalInput")       # rolled rb adj, last 512
    rwbr = nc.dram_tensor("rwbr", [NL, P, NDC], F32, kind="ExternalInput")
    ident_d = nc.dram_tensor("ident", [P, P], BF16, kind="ExternalInput")
    identf_d = nc.dram_tensor("identf", [P, P], F32, kind="ExternalInput")
    yout = nc.dram_tensor("y", [NQ, D], F32, kind="ExternalOutput")

    # internal DRAM
    sk = nc.dram_tensor("sk_buf", [H * NQ * SKR], F32)
    zd = nc.dram_tensor("zd_buf", [H * NQ], F32)
    agin = nc.dram_tensor("agin_buf", [D * NQ], BF16)
    agout = nc.dram_tensor("agout_buf", [2 * D, NQ], BF16)

    with tile.TileContext(nc) as tc, ExitStack() as stk:
        singles = stk.enter_context(tc.tile_pool(name="singles", bufs=1))
        wqp = stk.enter_context(tc.tile_pool(name="wqp", bufs=8))
        wTp = stk.enter_context(tc.tile_pool(name="wT", bufs=3))
        wrhsp = stk.enter_context(tc.tile_pool(name="wrhs", bufs=8))
        wop = stk.enter_context(tc.tile_pool(name="wo", bufs=8))
        gtp = stk.enter_context(tc.tile_pool(name="gt", bufs=16))
        atTp = stk.enter_context(tc.tile_pool(name="atT", bufs=8))
        bdtp = stk.enter_context(tc.tile_pool(name="bdt", bufs=5))
        brawp = stk.enter_context(tc.tile_pool(name="braw", bufs=2))
        repp = stk.enter_context(tc.tile_pool(name="rep", bufs=2))
        rbp = stk.enter_context(tc.tile_pool(name="rb", bufs=2))
        zrowp = stk.enter_context(tc.tile_pool(name="zrow", bufs=2))
        zrtp = stk.enter_context(tc.tile_pool(name="zrt", bufs=2))
        xwp = stk.enter_context(tc.tile_pool(name="xw", bufs=2))
        smallp = stk.enter_context(tc.tile_pool(name="small", bufs=16))
        psp = stk.enter_context(tc.tile_pool(name="psum", bufs=1,
                                             space="PSUM"))

        # ------------- persistent SBUF state -------------
        x_row = [singles.tile([P, D], F32, tag=f"xrow{i}", name=f"xrow{i}")
                 for i in range(NQC)]
        xTown = [singles.tile([P, NQ], BF16, tag=f"xto{i}", name=f"xto{i}")
                 for i in range(NDC)]       # my own x^T (this layer's in)
        xT = [singles.tile([P, 2 * NQ], BF16, tag=f"xt{i}", name=f"xt{i}")
              for i in range(NDC)]          # gathered x^T, my full batch
        xT1 = [singles.tile([P, NQ], BF16, tag=f"xt1_{i}", name=f"xt1_{i}")
               for i in range(NDC)]         # post-LN1 x^T, my rows
        kT = [singles.tile([P, S], BF16, tag=f"kt{i}", name=f"kt{i}")
              for i in range(NDC)]
        # v rows; head h occupies cols [h*65, h*65+64), col h*65+64 = WSC
        vrow = [singles.tile([P, H * 65], BF16, tag=f"vr{i}", name=f"vr{i}")
                for i in range(NDC)]
        rq = [singles.tile([P, NQ], BF16, tag=f"rq{i}", name=f"rq{i}")
              for i in range(NDC)]
        aoT = [singles.tile([P, NQ], BF16, tag=f"aoT{i}", name=f"aoT{i}")
               for i in range(NDC)]
        ident = singles.tile([P, P], BF16, tag="ident", name="ident")
        identf = singles.tile([P, P], F32, tag="identf", name="identf")
        eps_t = singles.tile([P, 1], F32, tag="eps", name="eps")
        zb_t = singles.tile([P, 1], F32, tag="zbias", name="zbias")
        b2_t = singles.tile([P, D], F32, tag="b2rep", name="b2rep")
        b1_t = singles.tile([P, NFC], F32, tag="b1", name="b1")
        rwb_t = singles.tile([P, NDC], F32, tag="rwb", name="rwb")
        zeros_t = singles.tile([P, P], F32, tag="zeros", name="zeros")

        nc.sync.dma_start(ident[:], ident_d.ap())
        nc.sync.dma_start(identf[:], identf_d.ap())
        nc.vector.memset(eps_t[:], EPS)
        nc.vector.memset(zb_t[:], 0.0)
        nc.vector.memset(zeros_t[:], 0.0)
        for dc in range(NDC):
            vv = vrow[dc].rearrange("p (h c) -> p h c", h=H)
            nc.vector.memset(vv[:, :, 64:65], WSC)
        skap = sk.ap()
        # zero skew pad cols [1024, 1152) once
        for blk in range(H * NQ // P):
            dst = _ap(skap, blk * P * SKR + 1024, [[SKR, P], [1, P]])
            nc.sync.dma_start(dst, zeros_t[:, :])
        for qc in range(NQC):
            nc.sync.dma_start(x_row[qc][:],
                              xrow0.ap()[qc * P:(qc + 1) * P, :])
        for dc in range(NDC):
            nc.sync.dma_start(xTown[dc][:],
                              xt0.ap()[dc * P:(dc + 1) * P, :])
        nc.sync.dma_start(agin.ap(), xt0.ap())

        def mm(out, lhsT, rhs, first=True, last=True):
            nc.tensor.matmul(out, lhsT, rhs, start=first, stop=last)

        def p_mm():
            return psp.tile([P, 512], F32, tag="mm", name="mm", bufs=2)

        def p_mm2():
            return psp.tile([P, 1024], F32, tag="mm2", name="mm2", bufs=2)

        def p_pt(dtype=BF16):
            # keep the slot at one PSUM bank (2KB) for either dtype
            pad = [P, 1024] if dtype == BF16 else [P, 512]
            return psp.tile([P, 512], dtype, tag="pt", name="pt", bufs=2,
                            padded_shape=pad)

        def layernorm(xr):
            st = smallp.tile([P, 2, 6], F32, tag="bnst", name="bnst")
            nc.vector.bn_stats(st[:, 0, :], xr[:, 0:512])
            nc.vector.bn_stats(st[:, 1, :], xr[:, 512:1024])
            mv = smallp.tile([P, 2], F32, tag="bnmv", name="bnmv")
            nc.vector.bn_aggr(mv[:], st[:])
            sd = smallp.tile([P, 1], F32, tag="sd", name="sd")
            nc.scalar.activation(sd[:], mv[:, 1:2], AF.Sqrt,
                                 bias=eps_t[:], scale=1.0)
            rs = smallp.tile([P, 1], F32, tag="rs", name="rs")
            nc.vector.reciprocal(rs[:], sd[:])
            nc.vector.tensor_scalar(
                out=xr[:], in0=xr[:], scalar1=mv[:, 0:1],
                scalar2=rs[:], op0=ALU.subtract, op1=ALU.mult)

        def transpose_to(dsts, src_bf16, qc):
            """src [128(q), 1024(d)] -> dsts[dc][:, qc*128:+128]."""
            for g in range(2):
                pt = p_pt()
                for k4 in range(4):
                    dc = g * 4 + k4
                    nc.tensor.transpose(
                        pt[:, k4 * P:(k4 + 1) * P],
                        src_bf16[:, dc * P:(dc + 1) * P], ident[:])
                for k4 in range(4):
                    dc = g * 4 + k4
                    nc.vector.tensor_copy(
                        dsts[dc][:, qc * P:(qc + 1) * P],
                        pt[:, k4 * P:(k4 + 1) * P])

        for li in range(NL):
            # ===== pairwise allgather x^T (async; overlapped by q-proj) ====
            nc.gpsimd.collective_compute(
                "AllGather", ALU.bypass,
                replica_groups=[[2 * g, 2 * g + 1]
                                for g in range(NCORES // 2)],
                ins=[agin.ap()], outs=[agout.ap()],
            )

            nc.sync.dma_start(rwb_t[:], rwbr.ap()[li])
            nc.sync.dma_start(b1_t[:], b1r.ap()[li])
            nc.sync.dma_start(
                b2_t[:], _ap(b2r.ap(), li * D, [[0, P], [1, D]]))

            # ===== q projection (own rows; AG-independent) =====
            for oc in range(NDC):
                wq = wqp.tile([P, D], FP8, tag="wq", name="wq")
                nc.sync.dma_start(wq[:], wproj.ap()[li, 0, oc])
                ps = p_mm()
                for dc in range(NDC):
                    mm(ps[:], wq[:, dc * P:(dc + 1) * P], xTown[dc][:],
                       first=(dc == 0), last=(dc == NDC - 1))
                nc.vector.tensor_scalar(
                    out=rq[oc][:], in0=ps[:],
                    scalar1=WSCI, scalar2=rwb_t[:, oc:oc + 1],
                    op0=ALU.mult, op1=ALU.add)

            # ===== gather my batch pair (waits on AG) =====
            for dc in range(NDC):
                for half in range(2):
                    nc.sync.dma_start(
                        xT[dc][:, half * NQ:(half + 1) * NQ],
                        agout.ap()[half * D + dc * P:
                                   half * D + (dc + 1) * P, :])

            # ===== k projection -> kT [dh, k] =====
            for oc in range(NDC):
                wk = wqp.tile([P, D], FP8, tag="wq", name="wq")
                nc.sync.dma_start(wk[:], wproj.ap()[li, 1, oc])
                psk = p_mm2()
                for jh in range(2):
                    for dc in range(NDC):
                        mm(psk[:, jh * 512:(jh + 1) * 512],
                           wk[:, dc * P:(dc + 1) * P],
                           xT[dc][:, jh * 512:(jh + 1) * 512],
                           first=(dc == 0), last=(dc == NDC - 1))
                nc.vector.tensor_scalar(
                    out=kT[oc][:], in0=psk[:], scalar1=WSCI, scalar2=None,
                    op0=ALU.mult)

            # ===== v projection -> vrow [k, dh] directly =====
            wvts = []
            for dc in range(NDC):
                w = wop.tile([P, D], FP8, tag="wo8", name="wvt")
                nc.sync.dma_start(w[:], wvt_d.ap()[li, dc])
                wvts.append(w)
            for kc in range(NDC):
                pv = p_mm2()
                for dc in range(NDC):
                    for half in range(2):
                        mm(pv[:, half * 512:(half + 1) * 512],
                           xT[dc][:, kc * P:(kc + 1) * P],
                           wvts[dc][:, half * 512:(half + 1) * 512],
                           first=(dc == 0), last=(dc == NDC - 1))
                vv = vrow[kc].rearrange("p (h c) -> p h c", h=H)
                sv = pv.rearrange("p (h c) -> p h c", c=64)
                nc.vector.tensor_scalar(
                    out=vv[:, :, 0:64], in0=sv[:],
                    scalar1=WSCI, scalar2=None, op0=ALU.mult)

            # ===== B~ scores + skew write (emitted per-head below) =====
            rept_c = {}

            def emit_b(h):
                ocn, rsub = h // 2, 64 * (h % 2)
                if h % 2 == 0:
                    rept = repp.tile([P, 512], BF16, tag="rep", name="rep")
                    nc.sync.dma_start(rept[:], repd.ap()[li, ocn])
                    rept_c[ocn] = rept
                rept = rept_c[ocn]
                rbt = rbp.tile([P, 512], BF16, tag="rb", name="rb")
                nc.sync.dma_start(
                    rbt[:], _ap(rbd.ap(), (li * H + h) * 512,
                                [[0, P], [1, 512]]))
                for qc in range(NQC):
                    w = (qc + 1) * P
                    psb = p_pt(F32)
                    mm(psb[:, 0:w],
                       rq[ocn][rsub:rsub + 64, qc * P:(qc + 1) * P],
                       rept[rsub:rsub + 64, 512 - w:512])
                    braw = brawp.tile([P, 512], F32, tag="braw", name="braw")
                    nc.vector.tensor_tensor(
                        braw[:, 0:w], psb[:, 0:w],
                        rbt[:, 512 - w:512], op=ALU.add)
                    base = h * NQ * SKR + qc * P * SKR + (896 - qc * P)
                    nc.sync.dma_start(
                        _ap(skap, base, [[SKR, P], [1, w]]),
                        braw[:, 0:w])

            # ===== attention (transposed scores [k, q]) =====
            def emit_scores(h):
                ocn, rsub = h // 2, 64 * (h % 2)
                bdts = []
                for qc in range(NQC):
                    w = (qc + 1) * P
                    bdt = bdtp.tile([P, 512], F32, tag="bdt", name="bdt")
                    base = h * NQ * SKR + qc * P * (SKR - 1) + 1023
                    nc.sync.dma_start(
                        bdt[:, 0:w],
                        _ap(skap, base, [[SKR - 1, P], [1, w]]))
                    bdts.append(bdt)
                ats = []
                for t in range(4):           # jc pair (2t, 2t+1)
                    pst = p_mm2()
                    for u in range(2):
                        jc = 2 * t + u
                        has_bd = jc < NQC
                        mm(pst[:, u * 512:(u + 1) * 512],
                           kT[ocn][rsub:rsub + 64, jc * P:(jc + 1) * P],
                           rq[ocn][rsub:rsub + 64, :],
                           first=True, last=not has_bd)
                    for u in range(2):
                        jc = 2 * t + u
                        if jc < NQC:
                            for qc in range(jc, NQC):
                                nc.tensor.matmul(
                                    pst[:, u * 512 + qc * P:
                                        u * 512 + (qc + 1) * P],
                                    bdts[qc][:, jc * P:(jc + 1) * P],
                                    identf[:], is_transpose=True,
                                    start=False, stop=(qc == NQC - 1))
                    at = atTp.tile([P, 1024], BF16, tag="atT", name="atT")
                    nc.scalar.activation(at[:], pst[:], AF.Exp,
                                         bias=zb_t[:], scale=SCALE)
                    ats.append(at)
                return ats

            def emit_av(h, ats):
                ocn, rsub = h // 2, 64 * (h % 2)
                # pav rows [0:65]: AV rows 0..63, Z (WSC-col sum) row 64
                pav = p_mm()
                for jc in range(NDC):
                    mm(pav[0:65, :],
                       vrow[jc][:, h * 65:h * 65 + 65],
                       ats[jc // 2][:, (jc % 2) * 512:(jc % 2 + 1) * 512],
                       first=(jc == 0), last=(jc == NDC - 1))
                zrow = zrowp.tile([P, 512], F32, tag="zrow", name="zrow")
                nc.vector.reciprocal(zrow[64:65, :], pav[64:65, :])
                nc.sync.dma_start(_ap(zd.ap(), h * NQ, [[1, NQ]]),
                                  zrow[64:65, :])
                zrt = zrtp.tile([64, 512], F32, tag="zrt", name="zrt")
                nc.sync.dma_start(
                    zrt[:], _ap(zd.ap(), h * NQ, [[0, 64], [1, NQ]]))
                tmpo = zrowp.tile([64, 512], BF16, tag="tmpo", name="tmpo")
                nc.vector.tensor_tensor(tmpo[:], pav[0:64, :], zrt[:],
                                        op=ALU.mult)
                nc.sync.dma_start(aoT[ocn][rsub:rsub + 64, :], tmpo[:])

            emit_b(0)
            emit_b(1)
            prev = None
            for h in range(H):
                if h + 2 < H:
                    emit_b(h + 2)
                ats = emit_scores(h)
                if prev is not None:
                    emit_av(h - 1, prev)
                prev = ats
            emit_av(H - 1, prev)

            # ===== Wo + residual + LN1 + xT1 =====
            wo_t = []
            for dc in range(NDC):
                w = wop.tile([P, D], FP8, tag="wo8", name="wo8")
                nc.sync.dma_start(
                    w[:], wot.ap()[li, dc * P:(dc + 1) * P, :])
                wo_t.append(w)

            def wo_finish(qc, pp):
                nc.vector.tensor_add(x_row[qc][:], x_row[qc][:], pp[:])
                layernorm(x_row[qc])
                xb = xwp.tile([P, D], BF16, tag="xb", name="xb")
                nc.vector.tensor_copy(xb[:], x_row[qc][:])
                transpose_to(xT1, xb, qc)

            pend = None
            for qc in range(NQC):
                pp = p_mm2()
                for dc in range(NDC):
                    for o2 in range(2):
                        mm(pp[:, o2 * 512:(o2 + 1) * 512],
                           aoT[dc][:, qc * P:(qc + 1) * P],
                           wo_t[dc][:, o2 * 512:(o2 + 1) * 512],
                           first=(dc == 0), last=(dc == NDC - 1))
                if pend is not None:
                    wo_finish(*pend)
                pend = (qc, pp)
            wo_finish(*pend)

            # b2 pre-add into residual stream
            for qc in range(NQC):
                nc.vector.tensor_add(x_row[qc][:], x_row[qc][:], b2_t[:])

            # ===== FFN =====
            for fh in range(2):
                gts = []
                for fc16 in range(16):
                    fc = fh * 16 + fc16
                    w1t_ = wTp.tile([P, D], BF16, tag="w1", name="w1")
                    nc.sync.dma_start(w1t_[:], w1r.ap()[li, fc])
                    ph = p_mm()
                    for dc in range(NDC):
                        mm(ph[:], w1t_[:, dc * P:(dc + 1) * P],
                           xT1[dc][:],
                           first=(dc == 0), last=(dc == NDC - 1))
                    gt = gtp.tile([P, NQ], BF16, tag="gt", name="gt")
                    nc.scalar.activation(gt[:], ph[:], AF.Gelu,
                                         bias=b1_t[:, fc:fc + 1],
                                         scale=1.0)
                    gts.append(gt)
                for fcg in range(2):
                    last_group = (fh == 1 and fcg == 1)
                    w2_t = []
                    for f8 in range(8):
                        w = wrhsp.tile([P, D], BF16, tag="wrhs",
                                       name="wrhs")
                        fc = fh * 16 + fcg * 8 + f8
                        nc.sync.dma_start(
                            w[:], w2t.ap()[li, fc * P:(fc + 1) * P, :])
                        w2_t.append(w)
                    for qc in range(NQC):
                        pf = p_mm2()
                        for o2 in range(2):
                            for f8 in range(8):
                                mm(pf[:, o2 * 512:(o2 + 1) * 512],
                                   gts[fcg * 8 + f8][:,
                                                     qc * P:(qc + 1) * P],
                                   w2_t[f8][:, o2 * 512:(o2 + 1) * 512],
                                   first=(f8 == 0), last=(f8 == 7))
                        nc.vector.tensor_add(x_row[qc][:], x_row[qc][:],
                                             pf[:])
                        if last_group:
                            # LN2 for this qc while later qcs' matmuls run
                            layernorm(x_row[qc])
                            if li == NL - 1:
                                nc.sync.dma_start(
                                    yout.ap()[qc * P:(qc + 1) * P, :],
                                    x_row[qc][:])
                            else:
                                xb = xwp.tile([P, D], BF16, tag="xb",
                                              name="xb")
                                nc.vector.tensor_copy(xb[:], x_row[qc][:])
                                transpose_to(xTown, xb, qc)
            if li < NL - 1:
                for dc in range(NDC):
                    nc.sync.dma_start(
                        _ap(agin.ap(), dc * P * NQ, [[NQ, P], [1, NQ]]),
                        xTown[dc][:])

    nc.finalize()
    return nc


def _prep_host(inputs):
    bf = ml_dtypes.bfloat16
    f8 = ml_dtypes.float8_e3m4
    embed = np.asarray(inputs["embed"], np.float32)
    seq = np.asarray(inputs["seq"]).astype(np.int64)
    x0 = embed[seq]                                   # [B, S, D] f32

    Wq = np.asarray(inputs["Wq"], np.float32)
    Wk = np.asarray(inputs["Wk"], np.float32)
    Wv = np.asarray(inputs["Wv"], np.float32)
    Wo = np.asarray(inputs["Wo"], np.float32)
    w1 = np.asarray(inputs["w1"], np.float32)
    w2 = np.asarray(inputs["w2"], np.float32)
    b1 = np.asarray(inputs["b1"], np.float32)
    b2 = np.asarray(inputs["b2"], np.float32)
    r_emb = np.asarray(inputs["r_emb"], np.float32)
    r_w_bias = np.asarray(inputs["r_w_bias"], np.float32)
    r_bias = np.asarray(inputs["r_bias"], np.float32)

    def packl(WT):   # [D, D] -> [NDC, P, D] lhsT pack
        return np.ascontiguousarray(
            WT.reshape(NDC, P, NDC, P).transpose(2, 1, 0, 3)
            .reshape(NDC, P, D))

    wproj = (np.stack([
        np.stack([packl(Wq[l].T), packl(Wk[l].T)])
        for l in range(NL)]) * WSC).astype(f8)
    wvt = (np.stack([Wv[l].T.reshape(NDC, P, D)
                     for l in range(NL)]) * WSC).astype(f8)
    wot = (np.stack([Wo[l].T for l in range(NL)]) * WSC).astype(f8)
    w1r = np.stack([
        np.ascontiguousarray(
            w1[l].T.reshape(NDC, P, NFC, P).transpose(2, 1, 0, 3)
            .reshape(NFC, P, D))
        for l in range(NL)]).astype(bf)
    w2t = np.stack([w2[l].T for l in range(NL)]).astype(bf)
    b1r = np.stack([b1[l].reshape(NFC, P).T for l in range(NL)])
    b1r = np.ascontiguousarray(b1r).astype(np.float32)
    b2r = b2.astype(np.float32)
    rwbr = np.stack([r_w_bias[l].reshape(D).reshape(NDC, P).T
                     for l in range(NL)])
    rwbr = np.ascontiguousarray(rwbr).astype(np.float32)

    # rep: per head-pair stacked re^T; rb_adj = rb - rwb @ re^T separately
    off = r_emb.shape[2] - S     # MAX_KLEN - S
    rep = np.empty((NL, NDC, P, S), np.float32)
    rba = np.empty((NL, H, S), np.float32)
    for l in range(NL):
        for h in range(H):
            re = r_emb[l, h, off:, :]            # [S, DH]
            rep[l, h // 2, (h % 2) * 64:(h % 2) * 64 + 64] = re.T
            rba[l, h] = r_bias[l, h, off:] - r_w_bias[l, h] @ re.T

    ident = np.eye(P, dtype=bf)
    identf = np.eye(P, dtype=np.float32)

    in_maps = []
    for c in range(NCORES):
        b, half = c // 2, c % 2
        q0 = half * NQ
        xr = np.ascontiguousarray(x0[b, q0:q0 + NQ]).astype(np.float32)
        xt = np.ascontiguousarray(x0[b, q0:q0 + NQ].T).astype(bf)
        repc = np.roll(rep, q0, axis=-1) if q0 else rep
        rbac = np.roll(rba, q0, axis=-1) if q0 else rba
        repc = np.ascontiguousarray(repc[..., 512:]).astype(bf)
        rbac = np.ascontiguousarray(rbac[..., 512:]).astype(bf)
        in_maps.append({
            "xrow0": xr, "xt0": xt, "wproj": wproj, "wvt": wvt, "wot": wot,
            "w1r": w1r, "w2t": w2t, "b1r": b1r, "b2r": b2r,
            "repd": repc, "rbd": rbac, "rwbr": rwbr, "ident": ident,
            "identf": identf,
        })
    return in_maps


def run(inputs, trace=False):
    if "nc" not in _cache:
        _cache["nc"] = build()
    nc = _cache["nc"]
    in_maps = _prep_host(inputs)
    res = run_bass_kernel_spmd(nc, in_maps, list(range(NCORES)),
                               trace=trace)
    y = np.zeros((B, S, D), np.float32)
    for c in range(NCORES):
        b, half = c // 2, c % 2
        y[b, half * NQ:(half + 1) * NQ] = res.results[c]["y"]
    return y, res


def kernel(**inputs) -> np.ndarray:
    y, _ = run(inputs)
    return y
